# revision 6
# baseline (speedup 1.0000x reference)
"""GCN edge-logits kernel for Trainium2 (8 NeuronCores, SPMD).

Structure: 2-layer GCN (PyG GCNConv with self-loops) + edge dot-product
scoring, N=1M nodes, E=16M edges.

Device strategy (per the edge-parallel sharding hint):
 - Edges are sharded across 8 cores by dst range (125K nodes/core).
 - Per core, own nodes are ordered by in-degree (desc) and each node's
   incoming edges occupy a fixed power-of-two slot block (64/32/16 slots)
   => message aggregation is a dense log2 strided-add reduction on device,
   and dst-side expansion is a dense doubling broadcast. No indirect
   addressing is needed on device for either.
 - The only irregular op — gathering node features u[src]/h1u[src]/h2[src]
   per edge slot — is performed on the host between the 4 device launches
   (np.take with host-precomputed static slot->src index maps).
 - All floating-point math (normalization, both GCN layers, ReLU, edge
   dots) runs on device.
"""
import os
import numpy as np

import concourse.bass as bass
import concourse.bacc as bacc
import concourse.mybir as mybir
import concourse.tile as tile
from concourse.bass_utils import run_bass_kernel_spmd

P = 128
N_NODES = 1_000_000
N_EDGES = 16_000_000
N_CORES = 8
OWN = N_NODES // N_CORES          # 125000
OWN_PAD = 125056                  # 128*977
RCOL = OWN_PAD // P               # 977
# degree classes: rank ranges (sorted by deg desc) -> slots per node
CLS = [
    (0, 128, 64),                 # 128 nodes x 64 slots
    (128, 55296, 32),             # 55168 nodes x 32 slots
    (55296, OWN_PAD, 16),         # 69760 nodes x 16 slots
]
TOT_SLOTS = sum((r1 - r0) * s for r0, r1, s in CLS)   # 2889728
C1 = TOT_SLOTS // P               # 22576
CHUNK = 1024                      # col chunk for slot-grid processing

F32 = mybir.dt.float32
I32 = mybir.dt.int32
BF16 = mybir.dt.bfloat16

LAST_EXEC_NS = []

_TRACE = bool(os.environ.get("BASS_GNN_TRACE"))
if _TRACE:
    # inline NTFF hook shim (the image's antenv lacks axon_hooks)
    import contextlib
    import ctypes
    import sys as _sys
    import types as _types

    def _install_shim():
        if "antenv.axon_hooks" in _sys.modules:
            return
        try:
            lib = ctypes.CDLL("/opt/axon/libaxon_pjrt.so")
            if not hasattr(lib, "axon_start_nrt_profile"):
                return
        except OSError:
            return
        lib.axon_start_nrt_profile.argtypes = [
            ctypes.POINTER(ctypes.c_int64), ctypes.c_size_t]
        lib.axon_start_nrt_profile.restype = ctypes.c_int64
        lib.axon_stop_nrt_profile.argtypes = [ctypes.c_char_p]
        lib.axon_stop_nrt_profile.restype = ctypes.c_int64

        @contextlib.contextmanager
        def _hook(output_dir, device_ids):
            import jax
            jax.devices()
            if device_ids:
                ids = (ctypes.c_int64 * len(device_ids))(*device_ids)
                rc = lib.axon_start_nrt_profile(ids, len(device_ids))
            else:
                rc = lib.axon_start_nrt_profile(None, 0)
            if rc != 0:
                raise RuntimeError(f"axon_start_nrt_profile rc={rc}")
            try:
                yield
            finally:
                n = lib.axon_stop_nrt_profile(str(output_dir).encode())
                if n < 0:
                    raise RuntimeError(f"axon_stop_nrt_profile rc={n}")

        mod = _types.ModuleType("antenv.axon_hooks")
        mod.get_axon_ntff_profile_hook = lambda: _hook
        mod.set_axon_ntff_profile_hook = lambda h: None
        _sys.modules["antenv.axon_hooks"] = mod

    _install_shim()


def _log_reduce(nc, t_ap, mm, S, d):
    """Reduce [128, mm, S, d] (d may be 1 => APs are [128, mm*S]) in place
    by halving adds; returns AP of [128, mm, 1, d] region (cols stride S*d)."""
    half = S
    while half > 1:
        half //= 2
        if d == 1:
            a = t_ap[:, :, 0:half]
            b = t_ap[:, :, half:2 * half]
        else:
            a = t_ap[:, :, 0:half, :]
            b = t_ap[:, :, half:2 * half, :]
        nc.vector.tensor_tensor(out=a, in0=a, in1=b, op=mybir.AluOpType.add)
    return t_ap


def _build_k1():
    """u = x * rsqrt(deg_in + 1) for a 125056-node linear shard."""
    nc = bacc.Bacc(None)
    x = nc.dram_tensor("x", [P, RCOL], F32, kind="ExternalInput")
    deg = nc.dram_tensor("deg", [P, RCOL], I32, kind="ExternalInput")
    u = nc.dram_tensor("u", [P, RCOL], F32, kind="ExternalOutput")
    with tile.TileContext(nc) as tc:
        with tc.tile_pool(name="sbuf", bufs=1) as sb:
            xt = sb.tile([P, RCOL], F32)
            dt = sb.tile([P, RCOL], I32)
            df = sb.tile([P, RCOL], F32)
            nc.sync.dma_start(out=xt[:], in_=x[:])
            nc.sync.dma_start(out=dt[:], in_=deg[:])
            nc.vector.tensor_copy(out=df[:], in_=dt[:])
            sq = sb.tile([P, RCOL], F32)
            nc.scalar.activation(sq[:], df[:],
                                 mybir.ActivationFunctionType.Sqrt,
                                 bias=1.0, scale=1.0)
            dinv = sb.tile([P, RCOL], F32)
            nc.vector.reciprocal(dinv[:], sq[:])
            ut = sb.tile([P, RCOL], F32)
            nc.vector.tensor_tensor(out=ut[:], in0=xt[:], in1=dinv[:],
                                    op=mybir.AluOpType.mult)
            nc.sync.dma_start(out=u[:], in_=ut[:])
    nc.compile()
    return nc


def _emit_dinv(nc, sb, deg_dram):
    dt = sb.tile([P, RCOL], I32)
    df = sb.tile([P, RCOL], F32)
    nc.sync.dma_start(out=dt[:], in_=deg_dram[:])
    nc.vector.tensor_copy(out=df[:], in_=dt[:])
    sq = sb.tile([P, RCOL], F32)
    nc.scalar.activation(sq[:], df[:], mybir.ActivationFunctionType.Sqrt,
                         bias=1.0, scale=1.0)
    dinv = sb.tile([P, RCOL], F32)
    nc.vector.reciprocal(dinv[:], sq[:])
    return dinv


def _emit_class_reduce(nc, sb, dram_pool, g_dram, d):
    """Reduce the slot grid g [P, C1(*d)] per node-class into agg_rank
    DRAM [OWN_PAD, d]-ish (rank-indexed). Returns the agg dram tile."""
    agg = dram_pool.tile([OWN_PAD * d], F32)
    colbase = 0
    for (r0, r1, S) in CLS:
        n = r1 - r0
        npp = n // P  # nodes per partition in this class
        mchunk = max(1, CHUNK // S)
        for m0 in range(0, npp, mchunk):
            mm = min(mchunk, npp - m0)
            t = sb.tile([P, mchunk * S * d], F32, tag=f"red{d}")
            src_ap = (g_dram[:, (colbase + m0 * S) * d:
                             (colbase + (m0 + mm) * S) * d]
                      if d > 1 else
                      g_dram[:, colbase + m0 * S: colbase + (m0 + mm) * S])
            nc.sync.dma_start(out=t[:, : mm * S * d], in_=src_ap)
            if d == 1:
                tv = t[:, : mm * S].rearrange("p (m s) -> p m s", m=mm)
            else:
                tv = t[:, : mm * S * d].rearrange(
                    "p (m s d) -> p m s d", m=mm, s=S)
            _log_reduce(nc, tv, mm, S, d)
            # result at stride S*d within t; write to agg at rank offset
            if d == 1:
                res_ap = t[:, : mm * S].rearrange(
                    "p (m s) -> p m s", m=mm)[:, :, 0:1]
            else:
                res_ap = t[:, : mm * S * d].rearrange(
                    "p (m s d) -> p m s d", m=mm, s=S)[:, :, 0:1, :]
            # dst AP into rank-indexed agg: partition stride npp*d, node stride d
            dst = bass.AP(agg[:].tensor, agg[:].offset + (r0 + m0) * d,
                          [[npp * d, P], [d, mm], [1, d]])
            nc.sync.dma_start(out=dst, in_=res_ap)
        colbase += npp * S
    return agg


def _build_k2():
    """layer1: agg over slots of g1=u[src]; h1=relu(W1*(dinv*agg+dinv^2*x)+b1);
    outputs h1u = h1*dinv and h1 (rank order)."""
    nc = bacc.Bacc(None)
    g1 = nc.dram_tensor("g1", [P, C1], F32, kind="ExternalInput")
    xr = nc.dram_tensor("xr", [P, RCOL], F32, kind="ExternalInput")
    degr = nc.dram_tensor("degr", [P, RCOL], I32, kind="ExternalInput")
    wvec = nc.dram_tensor("wvec", [28], F32, kind="ExternalInput")
    h1u = nc.dram_tensor("h1u", [P, RCOL * 4], F32, kind="ExternalOutput")
    h1o = nc.dram_tensor("h1o", [P, RCOL * 4], F32, kind="ExternalOutput")
    with tile.TileContext(nc) as tc:
        with (tc.tile_pool(name="sbuf", bufs=1) as sb,
              tc.tile_pool(name="stream", bufs=2) as st,
              tc.tile_pool(name="dram", bufs=1, space="DRAM") as dp):
            agg = _emit_class_reduce(nc, st, dp, g1, 1)
            dinv = _emit_dinv(nc, sb, degr)
            xt = sb.tile([P, RCOL], F32)
            nc.sync.dma_start(out=xt[:], in_=xr[:])
            aggr = sb.tile([P, RCOL], F32)
            nc.sync.dma_start(
                out=aggr[:],
                in_=bass.AP(agg[:].tensor, agg[:].offset, [[RCOL, P], [1, RCOL]]))
            wb = sb.tile([P, 28], F32)
            nc.sync.dma_start(out=wb[:], in_=wvec[None, :].to_broadcast([P, 28]))
            # pre = dinv*agg + dinv^2*x
            pre = sb.tile([P, RCOL], F32)
            nc.vector.tensor_tensor(out=pre[:], in0=aggr[:], in1=dinv[:],
                                    op=mybir.AluOpType.mult)
            t2 = sb.tile([P, RCOL], F32)
            nc.vector.tensor_tensor(out=t2[:], in0=xt[:], in1=dinv[:],
                                    op=mybir.AluOpType.mult)
            nc.vector.tensor_tensor(out=t2[:], in0=t2[:], in1=dinv[:],
                                    op=mybir.AluOpType.mult)
            nc.vector.tensor_tensor(out=pre[:], in0=pre[:], in1=t2[:],
                                    op=mybir.AluOpType.add)
            h1t = sb.tile([P, RCOL, 4], F32)
            h1ut = sb.tile([P, RCOL, 4], F32)
            for d in range(4):
                # relu(pre*W1[d] + b1[d])
                nc.scalar.activation(h1t[:, :, d], pre[:],
                                     mybir.ActivationFunctionType.Relu,
                                     bias=wb[:, 4 + d:5 + d],
                                     scale=wb[:, d:d + 1])
                nc.vector.tensor_tensor(out=h1ut[:, :, d], in0=h1t[:, :, d],
                                        in1=dinv[:], op=mybir.AluOpType.mult)
            nc.sync.dma_start(out=h1u[:], in_=h1ut[:])
            nc.sync.dma_start(out=h1o[:], in_=h1t[:])
    nc.compile()
    return nc


def _build_k3():
    """layer2: agg4 over slots of g2=h1u[src]; h2=(dinv*agg4+dinv^2*h1)@W2+b2."""
    nc = bacc.Bacc(None)
    g2 = nc.dram_tensor("g2", [P, C1 * 4], BF16, kind="ExternalInput")
    h1r = nc.dram_tensor("h1r", [P, RCOL * 4], F32, kind="ExternalInput")
    degr = nc.dram_tensor("degr", [P, RCOL], I32, kind="ExternalInput")
    wvec = nc.dram_tensor("wvec", [28], F32, kind="ExternalInput")
    h2o = nc.dram_tensor("h2o", [P, RCOL * 4], F32, kind="ExternalOutput")
    with tile.TileContext(nc) as tc:
        with (tc.tile_pool(name="sbuf", bufs=1) as sb,
              tc.tile_pool(name="stream", bufs=2) as st,
              tc.tile_pool(name="dram", bufs=1, space="DRAM") as dp):
            # bf16 -> f32 conversion happens chunk-wise inside reduce loads:
            # simplest: convert whole grid to f32 in DRAM first, chunked.
            gf = dp.tile([P, C1 * 4], F32)
            for c0 in range(0, C1, CHUNK):
                cc = min(CHUNK, C1 - c0)
                tb = st.tile([P, CHUNK * 4], BF16, tag="cvt_in")
                tf = st.tile([P, CHUNK * 4], F32, tag="cvt_out")
                nc.sync.dma_start(out=tb[:, : cc * 4],
                                  in_=g2[:, c0 * 4:(c0 + cc) * 4])
                nc.vector.tensor_copy(out=tf[:, : cc * 4], in_=tb[:, : cc * 4])
                nc.sync.dma_start(out=gf[:, c0 * 4:(c0 + cc) * 4],
                                  in_=tf[:, : cc * 4])
            agg = _emit_class_reduce(nc, st, dp, gf, 4)
            dinv = _emit_dinv(nc, sb, degr)
            h1t = sb.tile([P, RCOL, 4], F32)
            nc.sync.dma_start(out=h1t[:], in_=h1r[:])
            aggr = sb.tile([P, RCOL, 4], F32)
            nc.sync.dma_start(
                out=aggr[:],
                in_=bass.AP(agg[:].tensor, agg[:].offset,
                            [[RCOL * 4, P], [1, RCOL * 4]]))
            wb = sb.tile([P, 28], F32)
            nc.sync.dma_start(out=wb[:], in_=wvec[None, :].to_broadcast([P, 28]))
            dinv2 = sb.tile([P, RCOL], F32)
            nc.vector.tensor_tensor(out=dinv2[:], in0=dinv[:], in1=dinv[:],
                                    op=mybir.AluOpType.mult)
            z2 = sb.tile([P, RCOL, 4], F32)
            for d in range(4):
                nc.vector.tensor_tensor(out=z2[:, :, d], in0=aggr[:, :, d],
                                        in1=dinv[:], op=mybir.AluOpType.mult)
                t = sb.tile([P, RCOL], F32, tag="k3tmp")
                nc.vector.tensor_tensor(out=t[:], in0=h1t[:, :, d],
                                        in1=dinv2[:], op=mybir.AluOpType.mult)
                nc.vector.tensor_tensor(out=z2[:, :, d], in0=z2[:, :, d],
                                        in1=t[:], op=mybir.AluOpType.add)
            h2t = sb.tile([P, RCOL, 4], F32)
            for dout in range(4):
                # acc = z2_0*W2[0,dout]
                nc.vector.tensor_scalar(
                    out=h2t[:, :, dout], in0=z2[:, :, 0],
                    scalar1=wb[:, 8 + 0 * 4 + dout:8 + 0 * 4 + dout + 1],
                    scalar2=None, op0=mybir.AluOpType.mult)
                for din in range(1, 4):
                    nc.vector.scalar_tensor_tensor(
                        out=h2t[:, :, dout], in0=z2[:, :, din],
                        scalar=wb[:, 8 + din * 4 + dout:8 + din * 4 + dout + 1],
                        in1=h2t[:, :, dout],
                        op0=mybir.AluOpType.mult, op1=mybir.AluOpType.add)
                nc.vector.tensor_scalar(
                    out=h2t[:, :, dout], in0=h2t[:, :, dout],
                    scalar1=wb[:, 24 + dout:25 + dout], scalar2=None,
                    op0=mybir.AluOpType.add)
            nc.sync.dma_start(out=h2o[:], in_=h2t[:])
    nc.compile()
    return nc


def _build_k4():
    """logits per slot: dot(g3[slot,:], h2own[dst(slot),:]) with dense
    dst-expansion per class block."""
    nc = bacc.Bacc(None)
    g3 = nc.dram_tensor("g3", [P, C1 * 4], BF16, kind="ExternalInput")
    h2r = nc.dram_tensor("h2r", [P, RCOL * 4], F32, kind="ExternalInput")
    lg = nc.dram_tensor("lg", [P, C1], F32, kind="ExternalOutput")
    with tile.TileContext(nc) as tc:
        with (tc.tile_pool(name="sbuf", bufs=1) as sb,
              tc.tile_pool(name="stream", bufs=2) as st):
            h2t = sb.tile([P, RCOL, 4], F32)
            nc.sync.dma_start(out=h2t[:], in_=h2r[:])
            colbase = 0
            rank_base = 0
            for (r0, r1, S) in CLS:
                n = r1 - r0
                npp = n // P
                mchunk = max(1, CHUNK // S)
                for m0 in range(0, npp, mchunk):
                    mm = min(mchunk, npp - m0)
                    gb = st.tile([P, mchunk * S * 4], BF16, tag="g3in")
                    gfc = st.tile([P, mchunk * S * 4], F32, tag="g3f")
                    c0 = colbase + m0 * S
                    nc.sync.dma_start(out=gb[:, : mm * S * 4],
                                      in_=g3[:, c0 * 4:(c0 + mm * S) * 4])
                    nc.vector.tensor_copy(out=gfc[:, : mm * S * 4],
                                          in_=gb[:, : mm * S * 4])
                    # expand h2 of the mm nodes across their S slots:
                    # h2 rank mapping for this class: rank = r0 + p*npp + (m0+m)
                    ex = st.tile([P, mchunk * S * 4], F32, tag="expand")
                    exv = ex[:, : mm * S * 4].rearrange(
                        "p (m s d) -> p m s d", m=mm, s=S)
                    # seed slot 0 of each node; h2t is [P, RCOL(rank=p*RCOL+c), 4]
                    # class ranks r = r0 + p*npp + mq  -> h2 rank-layout uses
                    # r = p*RCOL + col; these differ, so reload class-mapped:
                    h2c = st.tile([P, mchunk, 1, 4], F32, tag="h2c")
                    src = bass.AP(h2r[:].tensor,
                                  h2r[:].offset + (r0 + m0) * 4,
                                  [[npp * 4, P], [4, mm], [1, 4]])
                    nc.sync.dma_start(out=h2c[:, : mm, :, :], in_=src)
                    nc.vector.tensor_copy(out=exv[:, :, 0:1, :],
                                          in_=h2c[:, : mm, :, :])
                    step = 1
                    while step < S:
                        w = min(step, S - step)
                        nc.vector.tensor_copy(out=exv[:, :, step:step + w, :],
                                              in_=exv[:, :, 0:w, :])
                        step += w
                    # dot: multiply then sum over d
                    nc.vector.tensor_tensor(out=gfc[:, : mm * S * 4],
                                            in0=gfc[:, : mm * S * 4],
                                            in1=ex[:, : mm * S * 4],
                                            op=mybir.AluOpType.mult)
                    gv = gfc[:, : mm * S * 4].rearrange(
                        "p (c d) -> p c d", d=4)
                    nc.vector.tensor_tensor(out=gv[:, :, 0:2], in0=gv[:, :, 0:2],
                                            in1=gv[:, :, 2:4],
                                            op=mybir.AluOpType.add)
                    nc.vector.tensor_tensor(out=gv[:, :, 0:1], in0=gv[:, :, 0:1],
                                            in1=gv[:, :, 1:2],
                                            op=mybir.AluOpType.add)
                    nc.sync.dma_start(out=lg[:, c0:c0 + mm * S],
                                      in_=gv[:, :, 0:1])
                colbase += npp * S
    nc.compile()
    return nc


_KERNELS = {}


def _get_kernels():
    if not _KERNELS:
        _KERNELS["k1"] = _build_k1()
        _KERNELS["k2"] = _build_k2()
        _KERNELS["k3"] = _build_k3()
        _KERNELS["k4"] = _build_k4()
    return _KERNELS


def _run(nc, in_maps):
    res = run_bass_kernel_spmd(nc, in_maps, list(range(N_CORES)),
                               trace=_TRACE)
    if res.exec_time_ns is not None:
        LAST_EXEC_NS.append(res.exec_time_ns)
    return res.results


def kernel(x, edge_index, W1, b1, W2, b2):
    x = np.asarray(x).reshape(-1).astype(np.float32)
    edge_index = np.asarray(edge_index)
    src = edge_index[0].astype(np.int64)
    dst = edge_index[1].astype(np.int64)
    import ml_dtypes

    LAST_EXEC_NS.clear()
    ks = _get_kernels()

    deg = np.bincount(dst, minlength=N_NODES).astype(np.int64)

    # ---- host index prep per core ----
    order_e = np.argsort(dst, kind="stable")
    dst_s = dst[order_e]
    src_s = src[order_e]
    bounds = np.searchsorted(dst_s, np.arange(N_CORES + 1) * OWN)

    x_pad = np.zeros(N_CORES * OWN_PAD, dtype=np.float32)
    deg_pad = np.zeros(N_CORES * OWN_PAD, dtype=np.int32)
    x_pad[:N_NODES] = x
    deg_pad[:N_NODES] = deg

    wvec = np.concatenate([
        np.asarray(W1, np.float32).reshape(-1),
        np.asarray(b1, np.float32).reshape(-1),
        np.asarray(W2, np.float32).reshape(-1),
        np.asarray(b2, np.float32).reshape(-1),
    ]).astype(np.float32)
    assert wvec.shape == (28,)

    cores = []
    for c in range(N_CORES):
        lo, hi = bounds[c], bounds[c + 1]
        sd = dst_s[lo:hi] - c * OWN      # local dst ids (sorted)
        ss = src_s[lo:hi]
        eid = order_e[lo:hi]

        d_own = np.zeros(OWN_PAD, dtype=np.int64)
        d_own[:OWN] = deg[c * OWN:(c + 1) * OWN]
        rank_order = np.argsort(-d_own, kind="stable")
        rank_of = np.empty(OWN_PAD, dtype=np.int64)
        rank_of[rank_order] = np.arange(OWN_PAD)

        dsr = np.sort(-d_own) * -1
        assert dsr[0] <= 64, f"deg {dsr[0]} exceeds max class"
        assert dsr[CLS[0][1]] <= 32, "class-32 boundary violated"
        assert dsr[CLS[1][1]] <= 16, "class-16 boundary violated"

        # flat-grid base address per rank
        base = np.empty(OWN_PAD, dtype=np.int64)
        colbase = 0
        for (r0, r1, S) in CLS:
            n = r1 - r0
            npp = n // P
            rr = np.arange(r0, r1)
            p = (rr - r0) // npp
            m = (rr - r0) % npp
            base[rr] = p * C1 + colbase + m * S
            colbase += npp * S

        # within-node edge position j (dst-sorted => runs contiguous)
        first = np.ones(len(sd), dtype=bool)
        first[1:] = sd[1:] != sd[:-1]
        runstart = np.maximum.accumulate(
            np.where(first, np.arange(len(sd)), 0))
        j = np.arange(len(sd)) - runstart

        slot = base[rank_of[sd]] + j
        src_slot = np.full(TOT_SLOTS, N_NODES, dtype=np.int64)
        src_slot[slot] = ss
        edge_of_slot = np.full(TOT_SLOTS, -1, dtype=np.int64)
        edge_of_slot[slot] = eid

        own_ids = c * OWN + rank_order  # rank -> original id (pad ids >= OWN are fake)
        own_valid = rank_order < OWN

        cores.append(dict(
            src_slot=src_slot, edge_of_slot=edge_of_slot,
            own_ids=own_ids, own_valid=own_valid,
            xr=x[np.minimum(own_ids, N_NODES - 1)].astype(np.float32)
            * own_valid,
            degr=(deg[np.minimum(own_ids, N_NODES - 1)] * own_valid
                  ).astype(np.int32),
        ))

    # ---- launch 1: u = x * rsqrt(deg+1) over all nodes (linear shards) ----
    in1 = [{"x": x_pad[c * OWN_PAD:(c + 1) * OWN_PAD].reshape(P, RCOL),
            "deg": deg_pad[c * OWN_PAD:(c + 1) * OWN_PAD].reshape(P, RCOL)}
           for c in range(N_CORES)]
    r1 = _run(ks["k1"], in1)
    u_full = np.zeros(N_CORES * OWN_PAD + 1, dtype=np.float32)
    for c in range(N_CORES):
        u_full[c * OWN_PAD:(c + 1) * OWN_PAD] = r1[c]["u"].reshape(-1)
    u_full[N_NODES:] = 0.0
    u_pad = np.zeros(N_NODES + 1, dtype=np.float32)
    u_pad[:N_NODES] = u_full[:N_NODES]

    # ---- launch 2: layer 1 ----
    in2 = []
    for c in range(N_CORES):
        g1 = u_pad[np.minimum(cores[c]["src_slot"], N_NODES)]
        in2.append({"g1": g1.reshape(P, C1).astype(np.float32),
                    "xr": cores[c]["xr"].reshape(P, RCOL),
                    "degr": cores[c]["degr"].reshape(P, RCOL),
                    "wvec": wvec})
    r2 = _run(ks["k2"], in2)
    h1u_full = np.zeros((N_NODES + 1, 4), dtype=np.float32)
    h1r_per_core = []
    for c in range(N_CORES):
        h1u_r = r2[c]["h1u"].reshape(OWN_PAD, 4)
        h1r_per_core.append(r2[c]["h1o"])
        ov = cores[c]["own_valid"]
        h1u_full[cores[c]["own_ids"][ov]] = h1u_r[ov]

    # ---- launch 3: layer 2 ----
    in3 = []
    for c in range(N_CORES):
        g2 = h1u_full[np.minimum(cores[c]["src_slot"], N_NODES)]
        in3.append({"g2": g2.reshape(P, C1 * 4).astype(ml_dtypes.bfloat16),
                    "h1r": h1r_per_core[c],
                    "degr": cores[c]["degr"].reshape(P, RCOL),
                    "wvec": wvec})
    r3 = _run(ks["k3"], in3)
    h2_full = np.zeros((N_NODES + 1, 4), dtype=np.float32)
    h2r_per_core = []
    for c in range(N_CORES):
        h2_r = r3[c]["h2o"].reshape(OWN_PAD, 4)
        h2r_per_core.append(r3[c]["h2o"])
        ov = cores[c]["own_valid"]
        h2_full[cores[c]["own_ids"][ov]] = h2_r[ov]

    # ---- launch 4: logits ----
    in4 = []
    for c in range(N_CORES):
        g3 = h2_full[np.minimum(cores[c]["src_slot"], N_NODES)]
        in4.append({"g3": g3.reshape(P, C1 * 4).astype(ml_dtypes.bfloat16),
                    "h2r": h2r_per_core[c]})
    r4 = _run(ks["k4"], in4)

    logits = np.zeros(N_EDGES, dtype=np.float32)
    for c in range(N_CORES):
        lg = r4[c]["lg"].reshape(-1)
        es = cores[c]["edge_of_slot"]
        valid = es >= 0
        logits[es[valid]] = lg[valid]
    return logits


# revision 7
# speedup vs baseline: 13.4618x; 13.4618x over previous
"""GCN edge-logits kernel for Trainium2 (8 NeuronCores, SPMD).

Structure: 2-layer GCN (PyG GCNConv with self-loops) + edge dot-product
scoring, N=1M nodes, E=16M edges.

Device strategy (per the edge-parallel sharding hint):
 - Edges are sharded across 8 cores by dst range (125K nodes/core).
 - Per core, own nodes are ordered by in-degree (desc) and each node's
   incoming edges occupy a fixed power-of-two slot block (64/32/16 slots)
   => message aggregation is a dense log2 strided-add reduction on device,
   and dst-side expansion is a dense doubling broadcast. No indirect
   addressing is needed on device for either.
 - The only irregular op — gathering node features u[src]/h1u[src]/h2[src]
   per edge slot — is performed on the host between the 4 device launches
   (np.take with host-precomputed static slot->src index maps).
 - All floating-point math (normalization, both GCN layers, ReLU, edge
   dots) runs on device.
"""
import os
import numpy as np

import concourse.bass as bass
import concourse.bacc as bacc
import concourse.mybir as mybir
import concourse.tile as tile
from concourse.bass_utils import run_bass_kernel_spmd

P = 128
N_NODES = 1_000_000
N_EDGES = 16_000_000
N_CORES = 8
OWN = N_NODES // N_CORES          # 125000
OWN_PAD = 125056                  # 128*977
RCOL = OWN_PAD // P               # 977
# degree classes: rank ranges (sorted by deg desc) -> slots per node
CLS = [
    (0, 128, 64),                 # 128 nodes x 64 slots
    (128, 55296, 32),             # 55168 nodes x 32 slots
    (55296, OWN_PAD, 16),         # 69760 nodes x 16 slots
]
TOT_SLOTS = sum((r1 - r0) * s for r0, r1, s in CLS)   # 2889728
C1 = TOT_SLOTS // P               # 22576
CHUNK = 1024                      # col chunk for slot-grid processing

F32 = mybir.dt.float32
I32 = mybir.dt.int32
BF16 = mybir.dt.bfloat16

LAST_EXEC_NS = []

_TRACE = bool(os.environ.get("BASS_GNN_TRACE"))
if _TRACE:
    # inline NTFF hook shim (the image's antenv lacks axon_hooks)
    import contextlib
    import ctypes
    import sys as _sys
    import types as _types

    def _install_shim():
        if "antenv.axon_hooks" in _sys.modules:
            return
        try:
            lib = ctypes.CDLL("/opt/axon/libaxon_pjrt.so")
            if not hasattr(lib, "axon_start_nrt_profile"):
                return
        except OSError:
            return
        lib.axon_start_nrt_profile.argtypes = [
            ctypes.POINTER(ctypes.c_int64), ctypes.c_size_t]
        lib.axon_start_nrt_profile.restype = ctypes.c_int64
        lib.axon_stop_nrt_profile.argtypes = [ctypes.c_char_p]
        lib.axon_stop_nrt_profile.restype = ctypes.c_int64

        @contextlib.contextmanager
        def _hook(output_dir, device_ids):
            import jax
            jax.devices()
            if device_ids:
                ids = (ctypes.c_int64 * len(device_ids))(*device_ids)
                rc = lib.axon_start_nrt_profile(ids, len(device_ids))
            else:
                rc = lib.axon_start_nrt_profile(None, 0)
            if rc != 0:
                raise RuntimeError(f"axon_start_nrt_profile rc={rc}")
            try:
                yield
            finally:
                n = lib.axon_stop_nrt_profile(str(output_dir).encode())
                if n < 0:
                    raise RuntimeError(f"axon_stop_nrt_profile rc={n}")

        mod = _types.ModuleType("antenv.axon_hooks")
        mod.get_axon_ntff_profile_hook = lambda: _hook
        mod.set_axon_ntff_profile_hook = lambda h: None
        _sys.modules["antenv.axon_hooks"] = mod

    _install_shim()


def _log_reduce(nc, t_ap, mm, S, d):
    """Reduce [128, mm, S, d] (d may be 1 => APs are [128, mm*S]) in place
    by halving adds; returns AP of [128, mm, 1, d] region (cols stride S*d)."""
    half = S
    while half > 1:
        half //= 2
        if d == 1:
            a = t_ap[:, :, 0:half]
            b = t_ap[:, :, half:2 * half]
        else:
            a = t_ap[:, :, 0:half, :]
            b = t_ap[:, :, half:2 * half, :]
        nc.vector.tensor_tensor(out=a, in0=a, in1=b, op=mybir.AluOpType.add)
    return t_ap


def _build_k1():
    """u = x * rsqrt(deg_in + 1) for a 125056-node linear shard."""
    nc = bacc.Bacc(None)
    x = nc.dram_tensor("x", [P, RCOL], F32, kind="ExternalInput")
    deg = nc.dram_tensor("deg", [P, RCOL], I32, kind="ExternalInput")
    u = nc.dram_tensor("u", [P, RCOL], F32, kind="ExternalOutput")
    with tile.TileContext(nc) as tc:
        with tc.tile_pool(name="sbuf", bufs=1) as sb:
            xt = sb.tile([P, RCOL], F32)
            dt = sb.tile([P, RCOL], I32)
            df = sb.tile([P, RCOL], F32)
            nc.sync.dma_start(out=xt[:], in_=x[:])
            nc.sync.dma_start(out=dt[:], in_=deg[:])
            nc.vector.tensor_copy(out=df[:], in_=dt[:])
            sq = sb.tile([P, RCOL], F32)
            nc.scalar.activation(sq[:], df[:],
                                 mybir.ActivationFunctionType.Sqrt,
                                 bias=1.0, scale=1.0)
            dinv = sb.tile([P, RCOL], F32)
            nc.vector.reciprocal(dinv[:], sq[:])
            ut = sb.tile([P, RCOL], F32)
            nc.vector.tensor_tensor(out=ut[:], in0=xt[:], in1=dinv[:],
                                    op=mybir.AluOpType.mult)
            nc.sync.dma_start(out=u[:], in_=ut[:])
    nc.compile()
    return nc


def _emit_dinv(nc, sb, deg_dram):
    dt = sb.tile([P, RCOL], I32)
    df = sb.tile([P, RCOL], F32)
    nc.sync.dma_start(out=dt[:], in_=deg_dram[:])
    nc.vector.tensor_copy(out=df[:], in_=dt[:])
    sq = sb.tile([P, RCOL], F32)
    nc.scalar.activation(sq[:], df[:], mybir.ActivationFunctionType.Sqrt,
                         bias=1.0, scale=1.0)
    dinv = sb.tile([P, RCOL], F32)
    nc.vector.reciprocal(dinv[:], sq[:])
    return dinv


def _emit_class_reduce(nc, sb, dram_pool, g_dram, d):
    """Reduce the slot grid g [P, C1(*d)] per node-class into agg_rank
    DRAM [OWN_PAD, d]-ish (rank-indexed). Returns the agg dram tile."""
    agg = dram_pool.tile([OWN_PAD * d], F32)
    colbase = 0
    for (r0, r1, S) in CLS:
        n = r1 - r0
        npp = n // P  # nodes per partition in this class
        mchunk = max(1, CHUNK // S)
        for m0 in range(0, npp, mchunk):
            mm = min(mchunk, npp - m0)
            t = sb.tile([P, mchunk * S * d], F32, tag=f"red{d}")
            src_ap = (g_dram[:, (colbase + m0 * S) * d:
                             (colbase + (m0 + mm) * S) * d]
                      if d > 1 else
                      g_dram[:, colbase + m0 * S: colbase + (m0 + mm) * S])
            nc.sync.dma_start(out=t[:, : mm * S * d], in_=src_ap)
            if d == 1:
                tv = t[:, : mm * S].rearrange("p (m s) -> p m s", m=mm)
            else:
                tv = t[:, : mm * S * d].rearrange(
                    "p (m s d) -> p m s d", m=mm, s=S)
            _log_reduce(nc, tv, mm, S, d)
            # result at stride S*d within t; write to agg at rank offset
            if d == 1:
                res_ap = t[:, : mm * S].rearrange(
                    "p (m s) -> p m s", m=mm)[:, :, 0:1]
            else:
                res_ap = t[:, : mm * S * d].rearrange(
                    "p (m s d) -> p m s d", m=mm, s=S)[:, :, 0:1, :]
            # dst AP into rank-indexed agg: partition stride npp*d, node stride d
            dst = bass.AP(agg[:].tensor, agg[:].offset + (r0 + m0) * d,
                          [[npp * d, P], [d, mm], [1, d]])
            nc.sync.dma_start(out=dst, in_=res_ap)
        colbase += npp * S
    return agg


def _build_k2():
    """layer1: agg over slots of g1=u[src]; h1=relu(W1*(dinv*agg+dinv^2*x)+b1);
    outputs h1u = h1*dinv and h1 (rank order)."""
    nc = bacc.Bacc(None)
    g1 = nc.dram_tensor("g1", [P, C1], F32, kind="ExternalInput")
    xr = nc.dram_tensor("xr", [P, RCOL], F32, kind="ExternalInput")
    degr = nc.dram_tensor("degr", [P, RCOL], I32, kind="ExternalInput")
    wvec = nc.dram_tensor("wvec", [28], F32, kind="ExternalInput")
    h1u = nc.dram_tensor("h1u", [P, RCOL * 4], F32, kind="ExternalOutput")
    h1o = nc.dram_tensor("h1o", [P, RCOL * 4], F32, kind="ExternalOutput")
    with tile.TileContext(nc) as tc:
        with (tc.tile_pool(name="sbuf", bufs=1) as sb,
              tc.tile_pool(name="stream", bufs=2) as st,
              tc.tile_pool(name="dram", bufs=1, space="DRAM") as dp):
            agg = _emit_class_reduce(nc, st, dp, g1, 1)
            dinv = _emit_dinv(nc, sb, degr)
            xt = sb.tile([P, RCOL], F32)
            nc.sync.dma_start(out=xt[:], in_=xr[:])
            aggr = sb.tile([P, RCOL], F32)
            nc.sync.dma_start(
                out=aggr[:],
                in_=bass.AP(agg[:].tensor, agg[:].offset, [[RCOL, P], [1, RCOL]]))
            wb = sb.tile([P, 28], F32)
            nc.sync.dma_start(out=wb[:], in_=wvec[None, :].to_broadcast([P, 28]))
            # pre = dinv*agg + dinv^2*x
            pre = sb.tile([P, RCOL], F32)
            nc.vector.tensor_tensor(out=pre[:], in0=aggr[:], in1=dinv[:],
                                    op=mybir.AluOpType.mult)
            t2 = sb.tile([P, RCOL], F32)
            nc.vector.tensor_tensor(out=t2[:], in0=xt[:], in1=dinv[:],
                                    op=mybir.AluOpType.mult)
            nc.vector.tensor_tensor(out=t2[:], in0=t2[:], in1=dinv[:],
                                    op=mybir.AluOpType.mult)
            nc.vector.tensor_tensor(out=pre[:], in0=pre[:], in1=t2[:],
                                    op=mybir.AluOpType.add)
            h1t = sb.tile([P, RCOL, 4], F32)
            h1ut = sb.tile([P, RCOL, 4], F32)
            for d in range(4):
                # relu(pre*W1[d] + b1[d])
                nc.scalar.activation(h1t[:, :, d], pre[:],
                                     mybir.ActivationFunctionType.Relu,
                                     bias=wb[:, 4 + d:5 + d],
                                     scale=wb[:, d:d + 1])
                nc.vector.tensor_tensor(out=h1ut[:, :, d], in0=h1t[:, :, d],
                                        in1=dinv[:], op=mybir.AluOpType.mult)
            nc.sync.dma_start(out=h1u[:], in_=h1ut[:])
            nc.sync.dma_start(out=h1o[:], in_=h1t[:])
    nc.compile()
    return nc


def _build_k3():
    """layer2: agg4 over slots of g2=h1u[src]; h2=(dinv*agg4+dinv^2*h1)@W2+b2."""
    nc = bacc.Bacc(None)
    g2 = nc.dram_tensor("g2", [P, C1 * 4], BF16, kind="ExternalInput")
    h1r = nc.dram_tensor("h1r", [P, RCOL * 4], F32, kind="ExternalInput")
    degr = nc.dram_tensor("degr", [P, RCOL], I32, kind="ExternalInput")
    wvec = nc.dram_tensor("wvec", [28], F32, kind="ExternalInput")
    h2o = nc.dram_tensor("h2o", [P, RCOL * 4], F32, kind="ExternalOutput")
    with tile.TileContext(nc) as tc:
        with (tc.tile_pool(name="sbuf", bufs=1) as sb,
              tc.tile_pool(name="stream", bufs=2) as st,
              tc.tile_pool(name="dram", bufs=1, space="DRAM") as dp):
            # bf16 -> f32 conversion happens chunk-wise inside reduce loads:
            # simplest: convert whole grid to f32 in DRAM first, chunked.
            gf = dp.tile([P, C1 * 4], F32)
            for c0 in range(0, C1, CHUNK):
                cc = min(CHUNK, C1 - c0)
                tb = st.tile([P, CHUNK * 4], BF16, tag="cvt_in")
                tf = st.tile([P, CHUNK * 4], F32, tag="cvt_out")
                nc.sync.dma_start(out=tb[:, : cc * 4],
                                  in_=g2[:, c0 * 4:(c0 + cc) * 4])
                nc.vector.tensor_copy(out=tf[:, : cc * 4], in_=tb[:, : cc * 4])
                nc.sync.dma_start(out=gf[:, c0 * 4:(c0 + cc) * 4],
                                  in_=tf[:, : cc * 4])
            agg = _emit_class_reduce(nc, st, dp, gf, 4)
            dinv = _emit_dinv(nc, sb, degr)
            h1t = sb.tile([P, RCOL, 4], F32)
            nc.sync.dma_start(out=h1t[:], in_=h1r[:])
            aggr = sb.tile([P, RCOL, 4], F32)
            nc.sync.dma_start(
                out=aggr[:],
                in_=bass.AP(agg[:].tensor, agg[:].offset,
                            [[RCOL * 4, P], [1, RCOL * 4]]))
            wb = sb.tile([P, 28], F32)
            nc.sync.dma_start(out=wb[:], in_=wvec[None, :].to_broadcast([P, 28]))
            dinv2 = sb.tile([P, RCOL], F32)
            nc.vector.tensor_tensor(out=dinv2[:], in0=dinv[:], in1=dinv[:],
                                    op=mybir.AluOpType.mult)
            z2 = sb.tile([P, RCOL, 4], F32)
            for d in range(4):
                nc.vector.tensor_tensor(out=z2[:, :, d], in0=aggr[:, :, d],
                                        in1=dinv[:], op=mybir.AluOpType.mult)
                t = sb.tile([P, RCOL], F32, tag="k3tmp")
                nc.vector.tensor_tensor(out=t[:], in0=h1t[:, :, d],
                                        in1=dinv2[:], op=mybir.AluOpType.mult)
                nc.vector.tensor_tensor(out=z2[:, :, d], in0=z2[:, :, d],
                                        in1=t[:], op=mybir.AluOpType.add)
            h2t = sb.tile([P, RCOL, 4], F32)
            for dout in range(4):
                # acc = z2_0*W2[0,dout]
                nc.vector.tensor_scalar(
                    out=h2t[:, :, dout], in0=z2[:, :, 0],
                    scalar1=wb[:, 8 + 0 * 4 + dout:8 + 0 * 4 + dout + 1],
                    scalar2=None, op0=mybir.AluOpType.mult)
                for din in range(1, 4):
                    nc.vector.scalar_tensor_tensor(
                        out=h2t[:, :, dout], in0=z2[:, :, din],
                        scalar=wb[:, 8 + din * 4 + dout:8 + din * 4 + dout + 1],
                        in1=h2t[:, :, dout],
                        op0=mybir.AluOpType.mult, op1=mybir.AluOpType.add)
                nc.vector.tensor_scalar(
                    out=h2t[:, :, dout], in0=h2t[:, :, dout],
                    scalar1=wb[:, 24 + dout:25 + dout], scalar2=None,
                    op0=mybir.AluOpType.add)
            nc.sync.dma_start(out=h2o[:], in_=h2t[:])
    nc.compile()
    return nc


def _build_k4():
    """logits per slot: dot(g3[slot,:], h2own[dst(slot),:]) with dense
    dst-expansion per class block."""
    nc = bacc.Bacc(None)
    g3 = nc.dram_tensor("g3", [P, C1 * 4], BF16, kind="ExternalInput")
    h2r = nc.dram_tensor("h2r", [P, RCOL * 4], F32, kind="ExternalInput")
    lg = nc.dram_tensor("lg", [P, C1], F32, kind="ExternalOutput")
    with tile.TileContext(nc) as tc:
        with (tc.tile_pool(name="sbuf", bufs=1) as sb,
              tc.tile_pool(name="stream", bufs=2) as st):
            h2t = sb.tile([P, RCOL, 4], F32)
            nc.sync.dma_start(out=h2t[:], in_=h2r[:])
            colbase = 0
            rank_base = 0
            for (r0, r1, S) in CLS:
                n = r1 - r0
                npp = n // P
                mchunk = max(1, CHUNK // S)
                for m0 in range(0, npp, mchunk):
                    mm = min(mchunk, npp - m0)
                    gb = st.tile([P, mchunk * S * 4], BF16, tag="g3in")
                    gfc = st.tile([P, mchunk * S * 4], F32, tag="g3f")
                    c0 = colbase + m0 * S
                    nc.sync.dma_start(out=gb[:, : mm * S * 4],
                                      in_=g3[:, c0 * 4:(c0 + mm * S) * 4])
                    nc.vector.tensor_copy(out=gfc[:, : mm * S * 4],
                                          in_=gb[:, : mm * S * 4])
                    # expand h2 of the mm nodes across their S slots:
                    # h2 rank mapping for this class: rank = r0 + p*npp + (m0+m)
                    ex = st.tile([P, mchunk * S * 4], F32, tag="expand")
                    exv = ex[:, : mm * S * 4].rearrange(
                        "p (m s d) -> p m s d", m=mm, s=S)
                    # seed slot 0 of each node; h2t is [P, RCOL(rank=p*RCOL+c), 4]
                    # class ranks r = r0 + p*npp + mq  -> h2 rank-layout uses
                    # r = p*RCOL + col; these differ, so reload class-mapped:
                    h2c = st.tile([P, mchunk, 1, 4], F32, tag="h2c")
                    src = bass.AP(h2r[:].tensor,
                                  h2r[:].offset + (r0 + m0) * 4,
                                  [[npp * 4, P], [4, mm], [1, 4]])
                    nc.sync.dma_start(out=h2c[:, : mm, :, :], in_=src)
                    nc.vector.tensor_copy(out=exv[:, :, 0:1, :],
                                          in_=h2c[:, : mm, :, :])
                    step = 1
                    while step < S:
                        w = min(step, S - step)
                        nc.vector.tensor_copy(out=exv[:, :, step:step + w, :],
                                              in_=exv[:, :, 0:w, :])
                        step += w
                    # dot: multiply then sum over d
                    nc.vector.tensor_tensor(out=gfc[:, : mm * S * 4],
                                            in0=gfc[:, : mm * S * 4],
                                            in1=ex[:, : mm * S * 4],
                                            op=mybir.AluOpType.mult)
                    gv = gfc[:, : mm * S * 4].rearrange(
                        "p (c d) -> p c d", d=4)
                    nc.vector.tensor_tensor(out=gv[:, :, 0:2], in0=gv[:, :, 0:2],
                                            in1=gv[:, :, 2:4],
                                            op=mybir.AluOpType.add)
                    nc.vector.tensor_tensor(out=gv[:, :, 0:1], in0=gv[:, :, 0:1],
                                            in1=gv[:, :, 1:2],
                                            op=mybir.AluOpType.add)
                    lt = st.tile([P, mchunk * S, 1], F32, tag="lout")
                    nc.vector.tensor_copy(out=lt[:, : mm * S, :],
                                          in_=gv[:, :, 0:1])
                    nc.sync.dma_start(out=lg[:, c0:c0 + mm * S],
                                      in_=lt[:, : mm * S, :])
                colbase += npp * S
    nc.compile()
    return nc


_KERNELS = {}


def _get_kernels():
    if not _KERNELS:
        _KERNELS["k1"] = _build_k1()
        _KERNELS["k2"] = _build_k2()
        _KERNELS["k3"] = _build_k3()
        _KERNELS["k4"] = _build_k4()
    return _KERNELS


def _run(nc, in_maps):
    res = run_bass_kernel_spmd(nc, in_maps, list(range(N_CORES)),
                               trace=_TRACE)
    if res.exec_time_ns is not None:
        LAST_EXEC_NS.append(res.exec_time_ns)
    return res.results


def kernel(x, edge_index, W1, b1, W2, b2):
    x = np.asarray(x).reshape(-1).astype(np.float32)
    edge_index = np.asarray(edge_index)
    src = edge_index[0].astype(np.int64)
    dst = edge_index[1].astype(np.int64)
    import ml_dtypes

    LAST_EXEC_NS.clear()
    ks = _get_kernels()

    deg = np.bincount(dst, minlength=N_NODES).astype(np.int64)

    # ---- host index prep per core ----
    order_e = np.argsort(dst, kind="stable")
    dst_s = dst[order_e]
    src_s = src[order_e]
    bounds = np.searchsorted(dst_s, np.arange(N_CORES + 1) * OWN)

    x_pad = np.zeros(N_CORES * OWN_PAD, dtype=np.float32)
    deg_pad = np.zeros(N_CORES * OWN_PAD, dtype=np.int32)
    x_pad[:N_NODES] = x
    deg_pad[:N_NODES] = deg

    wvec = np.concatenate([
        np.asarray(W1, np.float32).reshape(-1),
        np.asarray(b1, np.float32).reshape(-1),
        np.asarray(W2, np.float32).reshape(-1),
        np.asarray(b2, np.float32).reshape(-1),
    ]).astype(np.float32)
    assert wvec.shape == (28,)

    cores = []
    for c in range(N_CORES):
        lo, hi = bounds[c], bounds[c + 1]
        sd = dst_s[lo:hi] - c * OWN      # local dst ids (sorted)
        ss = src_s[lo:hi]
        eid = order_e[lo:hi]

        d_own = np.zeros(OWN_PAD, dtype=np.int64)
        d_own[:OWN] = deg[c * OWN:(c + 1) * OWN]
        rank_order = np.argsort(-d_own, kind="stable")
        rank_of = np.empty(OWN_PAD, dtype=np.int64)
        rank_of[rank_order] = np.arange(OWN_PAD)

        dsr = np.sort(-d_own) * -1
        assert dsr[0] <= 64, f"deg {dsr[0]} exceeds max class"
        assert dsr[CLS[0][1]] <= 32, "class-32 boundary violated"
        assert dsr[CLS[1][1]] <= 16, "class-16 boundary violated"

        # flat-grid base address per rank
        base = np.empty(OWN_PAD, dtype=np.int64)
        colbase = 0
        for (r0, r1, S) in CLS:
            n = r1 - r0
            npp = n // P
            rr = np.arange(r0, r1)
            p = (rr - r0) // npp
            m = (rr - r0) % npp
            base[rr] = p * C1 + colbase + m * S
            colbase += npp * S

        # within-node edge position j (dst-sorted => runs contiguous)
        first = np.ones(len(sd), dtype=bool)
        first[1:] = sd[1:] != sd[:-1]
        runstart = np.maximum.accumulate(
            np.where(first, np.arange(len(sd)), 0))
        j = np.arange(len(sd)) - runstart

        slot = base[rank_of[sd]] + j
        src_slot = np.full(TOT_SLOTS, N_NODES, dtype=np.int64)
        src_slot[slot] = ss
        edge_of_slot = np.full(TOT_SLOTS, -1, dtype=np.int64)
        edge_of_slot[slot] = eid

        own_ids = c * OWN + rank_order  # rank -> original id (pad ids >= OWN are fake)
        own_valid = rank_order < OWN

        cores.append(dict(
            src_slot=src_slot, edge_of_slot=edge_of_slot,
            own_ids=own_ids, own_valid=own_valid,
            xr=x[np.minimum(own_ids, N_NODES - 1)].astype(np.float32)
            * own_valid,
            degr=(deg[np.minimum(own_ids, N_NODES - 1)] * own_valid
                  ).astype(np.int32),
        ))

    # ---- launch 1: u = x * rsqrt(deg+1) over all nodes (linear shards) ----
    in1 = [{"x": x_pad[c * OWN_PAD:(c + 1) * OWN_PAD].reshape(P, RCOL),
            "deg": deg_pad[c * OWN_PAD:(c + 1) * OWN_PAD].reshape(P, RCOL)}
           for c in range(N_CORES)]
    r1 = _run(ks["k1"], in1)
    u_full = np.zeros(N_CORES * OWN_PAD + 1, dtype=np.float32)
    for c in range(N_CORES):
        u_full[c * OWN_PAD:(c + 1) * OWN_PAD] = r1[c]["u"].reshape(-1)
    u_full[N_NODES:] = 0.0
    u_pad = np.zeros(N_NODES + 1, dtype=np.float32)
    u_pad[:N_NODES] = u_full[:N_NODES]

    # ---- launch 2: layer 1 ----
    in2 = []
    for c in range(N_CORES):
        g1 = u_pad[np.minimum(cores[c]["src_slot"], N_NODES)]
        in2.append({"g1": g1.reshape(P, C1).astype(np.float32),
                    "xr": cores[c]["xr"].reshape(P, RCOL),
                    "degr": cores[c]["degr"].reshape(P, RCOL),
                    "wvec": wvec})
    r2 = _run(ks["k2"], in2)
    h1u_full = np.zeros((N_NODES + 1, 4), dtype=np.float32)
    h1r_per_core = []
    for c in range(N_CORES):
        h1u_r = r2[c]["h1u"].reshape(OWN_PAD, 4)
        h1r_per_core.append(r2[c]["h1o"])
        ov = cores[c]["own_valid"]
        h1u_full[cores[c]["own_ids"][ov]] = h1u_r[ov]

    # ---- launch 3: layer 2 ----
    in3 = []
    for c in range(N_CORES):
        g2 = h1u_full[np.minimum(cores[c]["src_slot"], N_NODES)]
        in3.append({"g2": g2.reshape(P, C1 * 4).astype(ml_dtypes.bfloat16),
                    "h1r": h1r_per_core[c],
                    "degr": cores[c]["degr"].reshape(P, RCOL),
                    "wvec": wvec})
    r3 = _run(ks["k3"], in3)
    h2_full = np.zeros((N_NODES + 1, 4), dtype=np.float32)
    h2r_per_core = []
    for c in range(N_CORES):
        h2_r = r3[c]["h2o"].reshape(OWN_PAD, 4)
        h2r_per_core.append(r3[c]["h2o"])
        ov = cores[c]["own_valid"]
        h2_full[cores[c]["own_ids"][ov]] = h2_r[ov]

    # ---- launch 4: logits ----
    in4 = []
    for c in range(N_CORES):
        g3 = h2_full[np.minimum(cores[c]["src_slot"], N_NODES)]
        in4.append({"g3": g3.reshape(P, C1 * 4).astype(ml_dtypes.bfloat16),
                    "h2r": h2r_per_core[c]})
    r4 = _run(ks["k4"], in4)

    logits = np.zeros(N_EDGES, dtype=np.float32)
    for c in range(N_CORES):
        lg = r4[c]["lg"].reshape(-1)
        es = cores[c]["edge_of_slot"]
        valid = es >= 0
        logits[es[valid]] = lg[valid]
    return logits


# revision 10
# speedup vs baseline: 48.7001x; 3.6176x over previous
"""GCN edge-logits kernel for Trainium2 (8 NeuronCores, SPMD).

Structure: 2-layer GCN (PyG GCNConv with self-loops) + edge dot-product
scoring, N=1M nodes, E=16M edges.

Device strategy (per the edge-parallel sharding hint):
 - Edges are sharded across 8 cores by dst range (125K nodes/core).
 - Per core, own nodes are ordered by in-degree (desc) and each node's
   incoming edges occupy a fixed power-of-two slot block (64/32/16 slots)
   => message aggregation is a dense log2 strided-add reduction on device,
   and dst-side expansion is a dense doubling broadcast. No indirect
   addressing is needed on device for either.
 - The only irregular op — gathering node features u[src]/h1u[src]/h2[src]
   per edge slot — is performed on the host between the 4 device launches
   (np.take with host-precomputed static slot->src index maps).
 - All floating-point math (normalization, both GCN layers, ReLU, edge
   dots) runs on device.
"""
import os
import numpy as np

import concourse.bass as bass
import concourse.bacc as bacc
import concourse.mybir as mybir
import concourse.tile as tile
from concourse.bass_utils import run_bass_kernel_spmd

P = 128
N_NODES = 1_000_000
N_EDGES = 16_000_000
N_CORES = 8
OWN = N_NODES // N_CORES          # 125000
OWN_PAD = 125056                  # 128*977
RCOL = OWN_PAD // P               # 977
# degree classes: rank ranges (sorted by deg desc) -> slots per node
CLS = [
    (0, 128, 64),                 # 128 nodes x 64 slots
    (128, 55296, 32),             # 55168 nodes x 32 slots
    (55296, OWN_PAD, 16),         # 69760 nodes x 16 slots
]
TOT_SLOTS = sum((r1 - r0) * s for r0, r1, s in CLS)   # 2889728
C1 = TOT_SLOTS // P               # 22576
CHUNK = 1024                      # col chunk for slot-grid processing

F32 = mybir.dt.float32
I32 = mybir.dt.int32
BF16 = mybir.dt.bfloat16

LAST_EXEC_NS = []

_TRACE = bool(os.environ.get("BASS_GNN_TRACE"))
if _TRACE:
    # inline NTFF hook shim (the image's antenv lacks axon_hooks)
    import contextlib
    import ctypes
    import sys as _sys
    import types as _types

    def _install_shim():
        if "antenv.axon_hooks" in _sys.modules:
            return
        try:
            lib = ctypes.CDLL("/opt/axon/libaxon_pjrt.so")
            if not hasattr(lib, "axon_start_nrt_profile"):
                return
        except OSError:
            return
        lib.axon_start_nrt_profile.argtypes = [
            ctypes.POINTER(ctypes.c_int64), ctypes.c_size_t]
        lib.axon_start_nrt_profile.restype = ctypes.c_int64
        lib.axon_stop_nrt_profile.argtypes = [ctypes.c_char_p]
        lib.axon_stop_nrt_profile.restype = ctypes.c_int64

        @contextlib.contextmanager
        def _hook(output_dir, device_ids):
            import jax
            jax.devices()
            if device_ids:
                ids = (ctypes.c_int64 * len(device_ids))(*device_ids)
                rc = lib.axon_start_nrt_profile(ids, len(device_ids))
            else:
                rc = lib.axon_start_nrt_profile(None, 0)
            if rc != 0:
                raise RuntimeError(f"axon_start_nrt_profile rc={rc}")
            try:
                yield
            finally:
                n = lib.axon_stop_nrt_profile(str(output_dir).encode())
                if n < 0:
                    raise RuntimeError(f"axon_stop_nrt_profile rc={n}")

        mod = _types.ModuleType("antenv.axon_hooks")
        mod.get_axon_ntff_profile_hook = lambda: _hook
        mod.set_axon_ntff_profile_hook = lambda h: None
        _sys.modules["antenv.axon_hooks"] = mod

    _install_shim()


def _log_reduce(nc, t_ap, mm, S, d):
    """Reduce [128, mm, S, d] (d may be 1 => APs are [128, mm*S]) in place
    by halving adds; returns AP of [128, mm, 1, d] region (cols stride S*d)."""
    half = S
    while half > 1:
        half //= 2
        if d == 1:
            a = t_ap[:, :, 0:half]
            b = t_ap[:, :, half:2 * half]
        else:
            a = t_ap[:, :, 0:half, :]
            b = t_ap[:, :, half:2 * half, :]
        nc.vector.tensor_tensor(out=a, in0=a, in1=b, op=mybir.AluOpType.add)
    return t_ap


def _build_k1():
    """u = x * rsqrt(deg_in + 1) for a 125056-node linear shard."""
    nc = bacc.Bacc(None)
    x = nc.dram_tensor("x", [P, RCOL], F32, kind="ExternalInput")
    deg = nc.dram_tensor("deg", [P, RCOL], I32, kind="ExternalInput")
    u = nc.dram_tensor("u", [P, RCOL], F32, kind="ExternalOutput")
    with tile.TileContext(nc) as tc:
        with tc.tile_pool(name="sbuf", bufs=1) as sb:
            xt = sb.tile([P, RCOL], F32)
            dt = sb.tile([P, RCOL], I32)
            df = sb.tile([P, RCOL], F32)
            nc.sync.dma_start(out=xt[:], in_=x[:])
            nc.sync.dma_start(out=dt[:], in_=deg[:])
            nc.vector.tensor_copy(out=df[:], in_=dt[:])
            sq = sb.tile([P, RCOL], F32)
            nc.scalar.activation(sq[:], df[:],
                                 mybir.ActivationFunctionType.Sqrt,
                                 bias=1.0, scale=1.0)
            dinv = sb.tile([P, RCOL], F32)
            nc.vector.reciprocal(dinv[:], sq[:])
            ut = sb.tile([P, RCOL], F32)
            nc.vector.tensor_tensor(out=ut[:], in0=xt[:], in1=dinv[:],
                                    op=mybir.AluOpType.mult)
            nc.sync.dma_start(out=u[:], in_=ut[:])
    nc.compile()
    return nc


def _emit_dinv(nc, sb, deg_dram):
    dt = sb.tile([P, RCOL], I32)
    df = sb.tile([P, RCOL], F32)
    nc.sync.dma_start(out=dt[:], in_=deg_dram[:])
    nc.vector.tensor_copy(out=df[:], in_=dt[:])
    sq = sb.tile([P, RCOL], F32)
    nc.scalar.activation(sq[:], df[:], mybir.ActivationFunctionType.Sqrt,
                         bias=1.0, scale=1.0)
    dinv = sb.tile([P, RCOL], F32)
    nc.vector.reciprocal(dinv[:], sq[:])
    return dinv


def _emit_class_reduce(nc, sb, dram_pool, g_dram, d, src_bf16=False):
    """Reduce the slot grid g [P, C1(*d)] per node-class into agg_rank
    DRAM [OWN_PAD, d]-ish (rank-indexed). Returns the agg dram tile.
    If src_bf16, the grid is bf16 and the first halving add also converts
    to f32 (fused, no DRAM round trip)."""
    agg = dram_pool.tile([OWN_PAD * d], F32)
    colbase = 0
    for (r0, r1, S) in CLS:
        n = r1 - r0
        npp = n // P  # nodes per partition in this class
        mchunk = max(1, CHUNK // S)
        for m0 in range(0, npp, mchunk):
            mm = min(mchunk, npp - m0)
            c_lo = (colbase + m0 * S) * d
            c_hi = (colbase + (m0 + mm) * S) * d
            if src_bf16:
                tb = sb.tile([P, mchunk * S * d], BF16, tag=f"redb{d}")
                nc.sync.dma_start(out=tb[:, : mm * S * d],
                                  in_=g_dram[:, c_lo:c_hi])
                S_eff = S // 2
                t = sb.tile([P, mchunk * S_eff * d], F32, tag=f"red{d}")
                if d == 1:
                    tbv = tb[:, : mm * S].rearrange("p (m s) -> p m s", m=mm)
                    tv = t[:, : mm * S_eff].rearrange(
                        "p (m s) -> p m s", m=mm)
                    nc.vector.tensor_tensor(
                        out=tv, in0=tbv[:, :, 0:S_eff],
                        in1=tbv[:, :, S_eff:S], op=mybir.AluOpType.add)
                else:
                    tbv = tb[:, : mm * S * d].rearrange(
                        "p (m s d) -> p m s d", m=mm, s=S)
                    tv = t[:, : mm * S_eff * d].rearrange(
                        "p (m s d) -> p m s d", m=mm, s=S_eff)
                    nc.vector.tensor_tensor(
                        out=tv, in0=tbv[:, :, 0:S_eff, :],
                        in1=tbv[:, :, S_eff:S, :], op=mybir.AluOpType.add)
            else:
                S_eff = S
                t = sb.tile([P, mchunk * S * d], F32, tag=f"red{d}")
                nc.sync.dma_start(out=t[:, : mm * S * d],
                                  in_=g_dram[:, c_lo:c_hi])
                if d == 1:
                    tv = t[:, : mm * S].rearrange("p (m s) -> p m s", m=mm)
                else:
                    tv = t[:, : mm * S * d].rearrange(
                        "p (m s d) -> p m s d", m=mm, s=S)
            _log_reduce(nc, tv, mm, S_eff, d)
            # compact the stride-S_eff*d results into a contiguous tile
            # before DMA (a strided-source DMA is descriptor-bound)
            cmp_t = sb.tile([P, mchunk * d], F32, tag=f"cmp{d}")
            if d == 1:
                res_ap = tv[:, :, 0:1]
                cmp_ap = cmp_t[:, : mm].rearrange("p (m o) -> p m o", o=1)
            else:
                res_ap = tv[:, :, 0:1, :]
                cmp_ap = cmp_t[:, : mm * d].rearrange(
                    "p (m o d) -> p m o d", o=1, d=d)
            nc.vector.tensor_copy(out=cmp_ap, in_=res_ap)
            # dst AP into rank-indexed agg: partition stride npp*d, then
            # a contiguous mm*d run per partition
            dst = bass.AP(agg[:].tensor, agg[:].offset + (r0 + m0) * d,
                          [[npp * d, P], [d, mm], [1, d]])
            nc.sync.dma_start(out=dst, in_=cmp_t[:, : mm * d])
        colbase += npp * S
    return agg


def _build_k2():
    """layer1: agg over slots of g1=u[src]; h1=relu(W1*(dinv*agg+dinv^2*x)+b1);
    outputs h1u = h1*dinv and h1 (rank order)."""
    nc = bacc.Bacc(None)
    g1 = nc.dram_tensor("g1", [P, C1], F32, kind="ExternalInput")
    xr = nc.dram_tensor("xr", [P, RCOL], F32, kind="ExternalInput")
    degr = nc.dram_tensor("degr", [P, RCOL], I32, kind="ExternalInput")
    wvec = nc.dram_tensor("wvec", [28], F32, kind="ExternalInput")
    h1u = nc.dram_tensor("h1u", [P, RCOL * 4], F32, kind="ExternalOutput")
    h1o = nc.dram_tensor("h1o", [P, RCOL * 4], F32, kind="ExternalOutput")
    with tile.TileContext(nc) as tc:
        with (tc.tile_pool(name="sbuf", bufs=1) as sb,
              tc.tile_pool(name="stream", bufs=2) as st,
              tc.tile_pool(name="dram", bufs=1, space="DRAM") as dp):
            agg = _emit_class_reduce(nc, st, dp, g1, 1)
            dinv = _emit_dinv(nc, sb, degr)
            xt = sb.tile([P, RCOL], F32)
            nc.sync.dma_start(out=xt[:], in_=xr[:])
            aggr = sb.tile([P, RCOL], F32)
            nc.sync.dma_start(
                out=aggr[:],
                in_=bass.AP(agg[:].tensor, agg[:].offset, [[RCOL, P], [1, RCOL]]))
            wb = sb.tile([P, 28], F32)
            nc.sync.dma_start(out=wb[:], in_=wvec[None, :].to_broadcast([P, 28]))
            # pre = dinv*agg + dinv^2*x
            pre = sb.tile([P, RCOL], F32)
            nc.vector.tensor_tensor(out=pre[:], in0=aggr[:], in1=dinv[:],
                                    op=mybir.AluOpType.mult)
            t2 = sb.tile([P, RCOL], F32)
            nc.vector.tensor_tensor(out=t2[:], in0=xt[:], in1=dinv[:],
                                    op=mybir.AluOpType.mult)
            nc.vector.tensor_tensor(out=t2[:], in0=t2[:], in1=dinv[:],
                                    op=mybir.AluOpType.mult)
            nc.vector.tensor_tensor(out=pre[:], in0=pre[:], in1=t2[:],
                                    op=mybir.AluOpType.add)
            h1t = sb.tile([P, RCOL, 4], F32)
            h1ut = sb.tile([P, RCOL, 4], F32)
            for d in range(4):
                # relu(pre*W1[d] + b1[d])
                nc.scalar.activation(h1t[:, :, d], pre[:],
                                     mybir.ActivationFunctionType.Relu,
                                     bias=wb[:, 4 + d:5 + d],
                                     scale=wb[:, d:d + 1])
                nc.vector.tensor_tensor(out=h1ut[:, :, d], in0=h1t[:, :, d],
                                        in1=dinv[:], op=mybir.AluOpType.mult)
            nc.sync.dma_start(out=h1u[:], in_=h1ut[:])
            nc.sync.dma_start(out=h1o[:], in_=h1t[:])
    nc.compile()
    return nc


def _build_k3():
    """layer2: agg4 over slots of g2=h1u[src]; h2=(dinv*agg4+dinv^2*h1)@W2+b2."""
    nc = bacc.Bacc(None)
    g2 = nc.dram_tensor("g2", [P, C1 * 4], BF16, kind="ExternalInput")
    h1r = nc.dram_tensor("h1r", [P, RCOL * 4], F32, kind="ExternalInput")
    degr = nc.dram_tensor("degr", [P, RCOL], I32, kind="ExternalInput")
    wvec = nc.dram_tensor("wvec", [28], F32, kind="ExternalInput")
    h2o = nc.dram_tensor("h2o", [P, RCOL * 4], F32, kind="ExternalOutput")
    with tile.TileContext(nc) as tc:
        with (tc.tile_pool(name="sbuf", bufs=1) as sb,
              tc.tile_pool(name="stream", bufs=2) as st,
              tc.tile_pool(name="dram", bufs=1, space="DRAM") as dp):
            agg = _emit_class_reduce(nc, st, dp, g2, 4, src_bf16=True)
            dinv = _emit_dinv(nc, sb, degr)
            h1t = sb.tile([P, RCOL, 4], F32)
            nc.sync.dma_start(out=h1t[:], in_=h1r[:])
            aggr = sb.tile([P, RCOL, 4], F32)
            nc.sync.dma_start(
                out=aggr[:],
                in_=bass.AP(agg[:].tensor, agg[:].offset,
                            [[RCOL * 4, P], [1, RCOL * 4]]))
            wb = sb.tile([P, 28], F32)
            nc.sync.dma_start(out=wb[:], in_=wvec[None, :].to_broadcast([P, 28]))
            dinv2 = sb.tile([P, RCOL], F32)
            nc.vector.tensor_tensor(out=dinv2[:], in0=dinv[:], in1=dinv[:],
                                    op=mybir.AluOpType.mult)
            z2 = sb.tile([P, RCOL, 4], F32)
            for d in range(4):
                nc.vector.tensor_tensor(out=z2[:, :, d], in0=aggr[:, :, d],
                                        in1=dinv[:], op=mybir.AluOpType.mult)
                t = sb.tile([P, RCOL], F32, tag="k3tmp")
                nc.vector.tensor_tensor(out=t[:], in0=h1t[:, :, d],
                                        in1=dinv2[:], op=mybir.AluOpType.mult)
                nc.vector.tensor_tensor(out=z2[:, :, d], in0=z2[:, :, d],
                                        in1=t[:], op=mybir.AluOpType.add)
            h2t = sb.tile([P, RCOL, 4], F32)
            for dout in range(4):
                # acc = z2_0*W2[0,dout]
                nc.vector.tensor_scalar(
                    out=h2t[:, :, dout], in0=z2[:, :, 0],
                    scalar1=wb[:, 8 + 0 * 4 + dout:8 + 0 * 4 + dout + 1],
                    scalar2=None, op0=mybir.AluOpType.mult)
                for din in range(1, 4):
                    nc.vector.scalar_tensor_tensor(
                        out=h2t[:, :, dout], in0=z2[:, :, din],
                        scalar=wb[:, 8 + din * 4 + dout:8 + din * 4 + dout + 1],
                        in1=h2t[:, :, dout],
                        op0=mybir.AluOpType.mult, op1=mybir.AluOpType.add)
                nc.vector.tensor_scalar(
                    out=h2t[:, :, dout], in0=h2t[:, :, dout],
                    scalar1=wb[:, 24 + dout:25 + dout], scalar2=None,
                    op0=mybir.AluOpType.add)
            nc.sync.dma_start(out=h2o[:], in_=h2t[:])
    nc.compile()
    return nc


def _build_k4():
    """logits per slot: dot(g3[slot,:], h2own[dst(slot),:]) with dense
    dst-expansion per class block."""
    nc = bacc.Bacc(None)
    g3 = nc.dram_tensor("g3", [P, C1 * 4], BF16, kind="ExternalInput")
    h2r = nc.dram_tensor("h2r", [P, RCOL * 4], F32, kind="ExternalInput")
    lg = nc.dram_tensor("lg", [P, C1], F32, kind="ExternalOutput")
    with tile.TileContext(nc) as tc:
        with (tc.tile_pool(name="sbuf", bufs=1) as sb,
              tc.tile_pool(name="stream", bufs=2) as st):
            colbase = 0
            for (r0, r1, S) in CLS:
                n = r1 - r0
                npp = n // P
                mchunk = max(1, CHUNK // S)
                for m0 in range(0, npp, mchunk):
                    mm = min(mchunk, npp - m0)
                    gb = st.tile([P, mchunk * S * 4], BF16, tag="g3in")
                    gfc = st.tile([P, mchunk * S * 4], F32, tag="g3f")
                    c0 = colbase + m0 * S
                    nc.sync.dma_start(out=gb[:, : mm * S * 4],
                                      in_=g3[:, c0 * 4:(c0 + mm * S) * 4])
                    # expand h2 of the mm nodes across their S slots (bf16);
                    # class rank mapping: rank = r0 + p*npp + (m0+m)
                    ex = st.tile([P, mchunk * S * 4], BF16, tag="expand")
                    exv = ex[:, : mm * S * 4].rearrange(
                        "p (m s d) -> p m s d", m=mm, s=S)
                    h2c = st.tile([P, mchunk, 1, 4], F32, tag="h2c")
                    src = bass.AP(h2r[:].tensor,
                                  h2r[:].offset + (r0 + m0) * 4,
                                  [[npp * 4, P], [4, mm], [1, 4]])
                    nc.sync.dma_start(out=h2c[:, : mm, :, :], in_=src)
                    nc.vector.tensor_copy(out=exv[:, :, 0:1, :],
                                          in_=h2c[:, : mm, :, :])
                    step = 1
                    while step < S:
                        w = min(step, S - step)
                        nc.vector.tensor_copy(out=exv[:, :, step:step + w, :],
                                              in_=exv[:, :, 0:w, :])
                        step += w
                    # dot: bf16*bf16 -> f32, then sum over d
                    nc.vector.tensor_tensor(out=gfc[:, : mm * S * 4],
                                            in0=gb[:, : mm * S * 4],
                                            in1=ex[:, : mm * S * 4],
                                            op=mybir.AluOpType.mult)
                    gv = gfc[:, : mm * S * 4].rearrange(
                        "p (c d) -> p c d", d=4)
                    nc.vector.tensor_tensor(out=gv[:, :, 0:2], in0=gv[:, :, 0:2],
                                            in1=gv[:, :, 2:4],
                                            op=mybir.AluOpType.add)
                    nc.vector.tensor_tensor(out=gv[:, :, 0:1], in0=gv[:, :, 0:1],
                                            in1=gv[:, :, 1:2],
                                            op=mybir.AluOpType.add)
                    lt = st.tile([P, mchunk * S, 1], F32, tag="lout")
                    nc.vector.tensor_copy(out=lt[:, : mm * S, :],
                                          in_=gv[:, :, 0:1])
                    nc.sync.dma_start(out=lg[:, c0:c0 + mm * S],
                                      in_=lt[:, : mm * S, :])
                colbase += npp * S
    nc.compile()
    return nc


_KERNELS = {}


def _get_kernels():
    if not _KERNELS:
        _KERNELS["k1"] = _build_k1()
        _KERNELS["k2"] = _build_k2()
        _KERNELS["k3"] = _build_k3()
        _KERNELS["k4"] = _build_k4()
    return _KERNELS


def _run(nc, in_maps):
    res = run_bass_kernel_spmd(nc, in_maps, list(range(N_CORES)),
                               trace=_TRACE)
    if res.exec_time_ns is not None:
        LAST_EXEC_NS.append(res.exec_time_ns)
    return res.results


def kernel(x, edge_index, W1, b1, W2, b2):
    x = np.asarray(x).reshape(-1).astype(np.float32)
    edge_index = np.asarray(edge_index)
    src = edge_index[0].astype(np.int64)
    dst = edge_index[1].astype(np.int64)
    import ml_dtypes

    LAST_EXEC_NS.clear()
    ks = _get_kernels()

    deg = np.bincount(dst, minlength=N_NODES).astype(np.int64)

    # ---- host index prep per core ----
    order_e = np.argsort(dst, kind="stable")
    dst_s = dst[order_e]
    src_s = src[order_e]
    bounds = np.searchsorted(dst_s, np.arange(N_CORES + 1) * OWN)

    x_pad = np.zeros(N_CORES * OWN_PAD, dtype=np.float32)
    deg_pad = np.zeros(N_CORES * OWN_PAD, dtype=np.int32)
    x_pad[:N_NODES] = x
    deg_pad[:N_NODES] = deg

    wvec = np.concatenate([
        np.asarray(W1, np.float32).reshape(-1),
        np.asarray(b1, np.float32).reshape(-1),
        np.asarray(W2, np.float32).reshape(-1),
        np.asarray(b2, np.float32).reshape(-1),
    ]).astype(np.float32)
    assert wvec.shape == (28,)

    cores = []
    for c in range(N_CORES):
        lo, hi = bounds[c], bounds[c + 1]
        sd = dst_s[lo:hi] - c * OWN      # local dst ids (sorted)
        ss = src_s[lo:hi]
        eid = order_e[lo:hi]

        d_own = np.zeros(OWN_PAD, dtype=np.int64)
        d_own[:OWN] = deg[c * OWN:(c + 1) * OWN]
        rank_order = np.argsort(-d_own, kind="stable")
        rank_of = np.empty(OWN_PAD, dtype=np.int64)
        rank_of[rank_order] = np.arange(OWN_PAD)

        dsr = np.sort(-d_own) * -1
        assert dsr[0] <= 64, f"deg {dsr[0]} exceeds max class"
        assert dsr[CLS[0][1]] <= 32, "class-32 boundary violated"
        assert dsr[CLS[1][1]] <= 16, "class-16 boundary violated"

        # flat-grid base address per rank
        base = np.empty(OWN_PAD, dtype=np.int64)
        colbase = 0
        for (r0, r1, S) in CLS:
            n = r1 - r0
            npp = n // P
            rr = np.arange(r0, r1)
            p = (rr - r0) // npp
            m = (rr - r0) % npp
            base[rr] = p * C1 + colbase + m * S
            colbase += npp * S

        # within-node edge position j (dst-sorted => runs contiguous)
        first = np.ones(len(sd), dtype=bool)
        first[1:] = sd[1:] != sd[:-1]
        runstart = np.maximum.accumulate(
            np.where(first, np.arange(len(sd)), 0))
        j = np.arange(len(sd)) - runstart

        slot = base[rank_of[sd]] + j
        src_slot = np.full(TOT_SLOTS, N_NODES, dtype=np.int64)
        src_slot[slot] = ss
        edge_of_slot = np.full(TOT_SLOTS, -1, dtype=np.int64)
        edge_of_slot[slot] = eid

        own_ids = c * OWN + rank_order  # rank -> original id (pad ids >= OWN are fake)
        own_valid = rank_order < OWN

        cores.append(dict(
            src_slot=src_slot, edge_of_slot=edge_of_slot,
            own_ids=own_ids, own_valid=own_valid,
            xr=x[np.minimum(own_ids, N_NODES - 1)].astype(np.float32)
            * own_valid,
            degr=(deg[np.minimum(own_ids, N_NODES - 1)] * own_valid
                  ).astype(np.int32),
        ))

    # ---- launch 1: u = x * rsqrt(deg+1) over all nodes (linear shards) ----
    in1 = [{"x": x_pad[c * OWN_PAD:(c + 1) * OWN_PAD].reshape(P, RCOL),
            "deg": deg_pad[c * OWN_PAD:(c + 1) * OWN_PAD].reshape(P, RCOL)}
           for c in range(N_CORES)]
    r1 = _run(ks["k1"], in1)
    u_full = np.zeros(N_CORES * OWN_PAD + 1, dtype=np.float32)
    for c in range(N_CORES):
        u_full[c * OWN_PAD:(c + 1) * OWN_PAD] = r1[c]["u"].reshape(-1)
    u_full[N_NODES:] = 0.0
    u_pad = np.zeros(N_NODES + 1, dtype=np.float32)
    u_pad[:N_NODES] = u_full[:N_NODES]

    # ---- launch 2: layer 1 ----
    in2 = []
    for c in range(N_CORES):
        g1 = u_pad[np.minimum(cores[c]["src_slot"], N_NODES)]
        in2.append({"g1": g1.reshape(P, C1).astype(np.float32),
                    "xr": cores[c]["xr"].reshape(P, RCOL),
                    "degr": cores[c]["degr"].reshape(P, RCOL),
                    "wvec": wvec})
    r2 = _run(ks["k2"], in2)
    h1u_full = np.zeros((N_NODES + 1, 4), dtype=np.float32)
    h1r_per_core = []
    for c in range(N_CORES):
        h1u_r = r2[c]["h1u"].reshape(OWN_PAD, 4)
        h1r_per_core.append(r2[c]["h1o"])
        ov = cores[c]["own_valid"]
        h1u_full[cores[c]["own_ids"][ov]] = h1u_r[ov]

    # ---- launch 3: layer 2 ----
    in3 = []
    for c in range(N_CORES):
        g2 = h1u_full[np.minimum(cores[c]["src_slot"], N_NODES)]
        in3.append({"g2": g2.reshape(P, C1 * 4).astype(ml_dtypes.bfloat16),
                    "h1r": h1r_per_core[c],
                    "degr": cores[c]["degr"].reshape(P, RCOL),
                    "wvec": wvec})
    r3 = _run(ks["k3"], in3)
    h2_full = np.zeros((N_NODES + 1, 4), dtype=np.float32)
    h2r_per_core = []
    for c in range(N_CORES):
        h2_r = r3[c]["h2o"].reshape(OWN_PAD, 4)
        h2r_per_core.append(r3[c]["h2o"])
        ov = cores[c]["own_valid"]
        h2_full[cores[c]["own_ids"][ov]] = h2_r[ov]

    # ---- launch 4: logits ----
    in4 = []
    for c in range(N_CORES):
        g3 = h2_full[np.minimum(cores[c]["src_slot"], N_NODES)]
        in4.append({"g3": g3.reshape(P, C1 * 4).astype(ml_dtypes.bfloat16),
                    "h2r": h2r_per_core[c]})
    r4 = _run(ks["k4"], in4)

    logits = np.zeros(N_EDGES, dtype=np.float32)
    for c in range(N_CORES):
        lg = r4[c]["lg"].reshape(-1)
        es = cores[c]["edge_of_slot"]
        valid = es >= 0
        logits[es[valid]] = lg[valid]
    return logits


# revision 14
# speedup vs baseline: 55.5025x; 1.1397x over previous
"""GCN edge-logits kernel for Trainium2 (8 NeuronCores, SPMD).

Structure: 2-layer GCN (PyG GCNConv with self-loops) + edge dot-product
scoring, N=1M nodes, E=16M edges.

Device strategy (per the edge-parallel sharding hint):
 - Edges are sharded across 8 cores by dst range (125K nodes/core).
 - Per core, own nodes are ordered by in-degree (desc) and each node's
   incoming edges occupy a fixed power-of-two slot block (64/32/16 slots)
   => message aggregation is a dense log2 strided-add reduction on device,
   and dst-side expansion is a dense doubling broadcast. No indirect
   addressing is needed on device for either.
 - The only irregular op — gathering node features u[src]/h1u[src]/h2[src]
   per edge slot — is performed on the host between the 4 device launches
   (np.take with host-precomputed static slot->src index maps).
 - All floating-point math (normalization, both GCN layers, ReLU, edge
   dots) runs on device.
"""
import os
import numpy as np

import concourse.bass as bass
import concourse.bacc as bacc
import concourse.mybir as mybir
import concourse.tile as tile
from concourse.bass_utils import run_bass_kernel_spmd

P = 128
N_NODES = 1_000_000
N_EDGES = 16_000_000
N_CORES = 8
OWN = N_NODES // N_CORES          # 125000
OWN_PAD = 125056                  # 128*977
RCOL = OWN_PAD // P               # 977
# degree classes: rank ranges (sorted by deg desc) -> slots per node
CLS = [
    (0, 128, 64),                 # 128 nodes x 64 slots
    (128, 55296, 32),             # 55168 nodes x 32 slots
    (55296, OWN_PAD, 16),         # 69760 nodes x 16 slots
]
TOT_SLOTS = sum((r1 - r0) * s for r0, r1, s in CLS)   # 2889728
C1 = TOT_SLOTS // P               # 22576
CHUNK = 2048                      # col chunk for slot-grid processing

F32 = mybir.dt.float32
I32 = mybir.dt.int32
BF16 = mybir.dt.bfloat16

LAST_EXEC_NS = []

_TRACE = bool(os.environ.get("BASS_GNN_TRACE"))
if _TRACE:
    # inline NTFF hook shim (the image's antenv lacks axon_hooks)
    import contextlib
    import ctypes
    import sys as _sys
    import types as _types

    def _install_shim():
        if "antenv.axon_hooks" in _sys.modules:
            return
        try:
            lib = ctypes.CDLL("/opt/axon/libaxon_pjrt.so")
            if not hasattr(lib, "axon_start_nrt_profile"):
                return
        except OSError:
            return
        lib.axon_start_nrt_profile.argtypes = [
            ctypes.POINTER(ctypes.c_int64), ctypes.c_size_t]
        lib.axon_start_nrt_profile.restype = ctypes.c_int64
        lib.axon_stop_nrt_profile.argtypes = [ctypes.c_char_p]
        lib.axon_stop_nrt_profile.restype = ctypes.c_int64

        @contextlib.contextmanager
        def _hook(output_dir, device_ids):
            import jax
            jax.devices()
            if device_ids:
                ids = (ctypes.c_int64 * len(device_ids))(*device_ids)
                rc = lib.axon_start_nrt_profile(ids, len(device_ids))
            else:
                rc = lib.axon_start_nrt_profile(None, 0)
            if rc != 0:
                raise RuntimeError(f"axon_start_nrt_profile rc={rc}")
            try:
                yield
            finally:
                n = lib.axon_stop_nrt_profile(str(output_dir).encode())
                if n < 0:
                    raise RuntimeError(f"axon_stop_nrt_profile rc={n}")

        mod = _types.ModuleType("antenv.axon_hooks")
        mod.get_axon_ntff_profile_hook = lambda: _hook
        mod.set_axon_ntff_profile_hook = lambda h: None
        _sys.modules["antenv.axon_hooks"] = mod

    _install_shim()


def _log_reduce(nc, t_ap, mm, S, d):
    """Reduce [128, mm, S, d] (d may be 1 => APs are [128, mm*S]) in place
    by halving adds; returns AP of [128, mm, 1, d] region (cols stride S*d)."""
    half = S
    while half > 1:
        half //= 2
        if d == 1:
            a = t_ap[:, :, 0:half]
            b = t_ap[:, :, half:2 * half]
        else:
            a = t_ap[:, :, 0:half, :]
            b = t_ap[:, :, half:2 * half, :]
        nc.vector.tensor_tensor(out=a, in0=a, in1=b, op=mybir.AluOpType.add)
    return t_ap


def _build_k1():
    """u = x * rsqrt(deg_in + 1) for a 125056-node linear shard."""
    nc = bacc.Bacc(None)
    x = nc.dram_tensor("x", [P, RCOL], F32, kind="ExternalInput")
    deg = nc.dram_tensor("deg", [P, RCOL], I32, kind="ExternalInput")
    u = nc.dram_tensor("u", [P, RCOL], F32, kind="ExternalOutput")
    with tile.TileContext(nc) as tc:
        with tc.tile_pool(name="sbuf", bufs=1) as sb:
            xt = sb.tile([P, RCOL], F32)
            dt = sb.tile([P, RCOL], I32)
            df = sb.tile([P, RCOL], F32)
            nc.sync.dma_start(out=xt[:], in_=x[:])
            nc.sync.dma_start(out=dt[:], in_=deg[:])
            nc.vector.tensor_copy(out=df[:], in_=dt[:])
            sq = sb.tile([P, RCOL], F32)
            nc.scalar.activation(sq[:], df[:],
                                 mybir.ActivationFunctionType.Sqrt,
                                 bias=1.0, scale=1.0)
            dinv = sb.tile([P, RCOL], F32)
            nc.vector.reciprocal(dinv[:], sq[:])
            ut = sb.tile([P, RCOL], F32)
            nc.vector.tensor_tensor(out=ut[:], in0=xt[:], in1=dinv[:],
                                    op=mybir.AluOpType.mult)
            nc.sync.dma_start(out=u[:], in_=ut[:])
    nc.compile()
    return nc


def _emit_dinv(nc, sb, deg_dram):
    dt = sb.tile([P, RCOL], I32)
    df = sb.tile([P, RCOL], F32)
    nc.sync.dma_start(out=dt[:], in_=deg_dram[:])
    nc.vector.tensor_copy(out=df[:], in_=dt[:])
    sq = sb.tile([P, RCOL], F32)
    nc.scalar.activation(sq[:], df[:], mybir.ActivationFunctionType.Sqrt,
                         bias=1.0, scale=1.0)
    dinv = sb.tile([P, RCOL], F32)
    nc.vector.reciprocal(dinv[:], sq[:])
    return dinv


def _emit_class_reduce(nc, sb, dram_pool, g_dram, d, src_bf16=False):
    """Reduce the slot grid g [P, C1(*d)] per node-class into agg_rank
    DRAM [OWN_PAD, d]-ish (rank-indexed). Returns the agg dram tile.
    If src_bf16, the grid is bf16 and the first halving add also converts
    to f32 (fused, no DRAM round trip)."""
    agg = dram_pool.tile([OWN_PAD * d], F32)
    colbase = 0
    for (r0, r1, S) in CLS:
        n = r1 - r0
        npp = n // P  # nodes per partition in this class
        mchunk = max(1, CHUNK // S)
        for m0 in range(0, npp, mchunk):
            mm = min(mchunk, npp - m0)
            c_lo = (colbase + m0 * S) * d
            c_hi = (colbase + (m0 + mm) * S) * d
            if src_bf16:
                tb = sb.tile([P, mchunk * S * d], BF16, tag=f"redb{d}")
                nc.sync.dma_start(out=tb[:, : mm * S * d],
                                  in_=g_dram[:, c_lo:c_hi])
                S_eff = S // 2
                t = sb.tile([P, mchunk * S_eff * d], F32, tag=f"red{d}")
                if d == 1:
                    tbv = tb[:, : mm * S].rearrange("p (m s) -> p m s", m=mm)
                    tv = t[:, : mm * S_eff].rearrange(
                        "p (m s) -> p m s", m=mm)
                    nc.vector.tensor_tensor(
                        out=tv, in0=tbv[:, :, 0:S_eff],
                        in1=tbv[:, :, S_eff:S], op=mybir.AluOpType.add)
                else:
                    tbv = tb[:, : mm * S * d].rearrange(
                        "p (m s d) -> p m s d", m=mm, s=S)
                    tv = t[:, : mm * S_eff * d].rearrange(
                        "p (m s d) -> p m s d", m=mm, s=S_eff)
                    nc.vector.tensor_tensor(
                        out=tv, in0=tbv[:, :, 0:S_eff, :],
                        in1=tbv[:, :, S_eff:S, :], op=mybir.AluOpType.add)
            else:
                S_eff = S
                t = sb.tile([P, mchunk * S * d], F32, tag=f"red{d}")
                nc.sync.dma_start(out=t[:, : mm * S * d],
                                  in_=g_dram[:, c_lo:c_hi])
                if d == 1:
                    tv = t[:, : mm * S].rearrange("p (m s) -> p m s", m=mm)
                else:
                    tv = t[:, : mm * S * d].rearrange(
                        "p (m s d) -> p m s d", m=mm, s=S)
            _log_reduce(nc, tv, mm, S_eff, d)
            # compact the stride-S_eff*d results into a contiguous tile
            # before DMA (a strided-source DMA is descriptor-bound)
            cmp_t = sb.tile([P, mchunk * d], F32, tag=f"cmp{d}")
            if d == 1:
                res_ap = tv[:, :, 0:1]
                cmp_ap = cmp_t[:, : mm].rearrange("p (m o) -> p m o", o=1)
            else:
                res_ap = tv[:, :, 0:1, :]
                cmp_ap = cmp_t[:, : mm * d].rearrange(
                    "p (m o d) -> p m o d", o=1, d=d)
            nc.vector.tensor_copy(out=cmp_ap, in_=res_ap)
            # dst AP into rank-indexed agg: partition stride npp*d, then
            # a contiguous mm*d run per partition
            dst = bass.AP(agg[:].tensor, agg[:].offset + (r0 + m0) * d,
                          [[npp * d, P], [d, mm], [1, d]])
            nc.sync.dma_start(out=dst, in_=cmp_t[:, : mm * d])
        colbase += npp * S
    return agg


def _build_k2():
    """layer1: agg over slots of g1=u[src]; h1=relu(W1*(dinv*agg+dinv^2*x)+b1);
    outputs h1u = h1*dinv and h1 (rank order)."""
    nc = bacc.Bacc(None)
    g1 = nc.dram_tensor("g1", [P, C1], BF16, kind="ExternalInput")
    xr = nc.dram_tensor("xr", [P, RCOL], F32, kind="ExternalInput")
    degr = nc.dram_tensor("degr", [P, RCOL], I32, kind="ExternalInput")
    wvec = nc.dram_tensor("wvec", [28], F32, kind="ExternalInput")
    h1u = nc.dram_tensor("h1u", [P, RCOL * 4], F32, kind="ExternalOutput")
    h1o = nc.dram_tensor("h1o", [P, RCOL * 4], F32, kind="ExternalOutput")
    with tile.TileContext(nc) as tc:
        with (tc.tile_pool(name="sbuf", bufs=1) as sb,
              tc.tile_pool(name="stream", bufs=2) as st,
              tc.tile_pool(name="dram", bufs=1, space="DRAM") as dp):
            agg = _emit_class_reduce(nc, st, dp, g1, 1, src_bf16=True)
            dinv = _emit_dinv(nc, sb, degr)
            xt = sb.tile([P, RCOL], F32)
            nc.sync.dma_start(out=xt[:], in_=xr[:])
            aggr = sb.tile([P, RCOL], F32)
            nc.sync.dma_start(
                out=aggr[:],
                in_=bass.AP(agg[:].tensor, agg[:].offset, [[RCOL, P], [1, RCOL]]))
            wb = sb.tile([P, 28], F32)
            nc.sync.dma_start(out=wb[:], in_=wvec[None, :].to_broadcast([P, 28]))
            # pre = dinv*agg + dinv^2*x
            pre = sb.tile([P, RCOL], F32)
            nc.vector.tensor_tensor(out=pre[:], in0=aggr[:], in1=dinv[:],
                                    op=mybir.AluOpType.mult)
            t2 = sb.tile([P, RCOL], F32)
            nc.vector.tensor_tensor(out=t2[:], in0=xt[:], in1=dinv[:],
                                    op=mybir.AluOpType.mult)
            nc.vector.tensor_tensor(out=t2[:], in0=t2[:], in1=dinv[:],
                                    op=mybir.AluOpType.mult)
            nc.vector.tensor_tensor(out=pre[:], in0=pre[:], in1=t2[:],
                                    op=mybir.AluOpType.add)
            h1t = sb.tile([P, RCOL, 4], F32)
            h1ut = sb.tile([P, RCOL, 4], F32)
            for d in range(4):
                # relu(pre*W1[d] + b1[d])
                nc.scalar.activation(h1t[:, :, d], pre[:],
                                     mybir.ActivationFunctionType.Relu,
                                     bias=wb[:, 4 + d:5 + d],
                                     scale=wb[:, d:d + 1])
                nc.vector.tensor_tensor(out=h1ut[:, :, d], in0=h1t[:, :, d],
                                        in1=dinv[:], op=mybir.AluOpType.mult)
            nc.sync.dma_start(out=h1u[:], in_=h1ut[:])
            nc.sync.dma_start(out=h1o[:], in_=h1t[:])
    nc.compile()
    return nc


def _build_k3():
    """layer2: agg4 over slots of g2=h1u[src]; h2=(dinv*agg4+dinv^2*h1)@W2+b2."""
    nc = bacc.Bacc(None)
    g2 = nc.dram_tensor("g2", [P, C1 * 4], BF16, kind="ExternalInput")
    h1r = nc.dram_tensor("h1r", [P, RCOL * 4], F32, kind="ExternalInput")
    degr = nc.dram_tensor("degr", [P, RCOL], I32, kind="ExternalInput")
    wvec = nc.dram_tensor("wvec", [28], F32, kind="ExternalInput")
    h2o = nc.dram_tensor("h2o", [P, RCOL * 4], F32, kind="ExternalOutput")
    with tile.TileContext(nc) as tc:
        with (tc.tile_pool(name="sbuf", bufs=1) as sb,
              tc.tile_pool(name="stream", bufs=2) as st,
              tc.tile_pool(name="dram", bufs=1, space="DRAM") as dp):
            agg = _emit_class_reduce(nc, st, dp, g2, 4, src_bf16=True)
            dinv = _emit_dinv(nc, sb, degr)
            h1t = sb.tile([P, RCOL, 4], F32)
            nc.sync.dma_start(out=h1t[:], in_=h1r[:])
            aggr = sb.tile([P, RCOL, 4], F32)
            nc.sync.dma_start(
                out=aggr[:],
                in_=bass.AP(agg[:].tensor, agg[:].offset,
                            [[RCOL * 4, P], [1, RCOL * 4]]))
            wb = sb.tile([P, 28], F32)
            nc.sync.dma_start(out=wb[:], in_=wvec[None, :].to_broadcast([P, 28]))
            dinv2 = sb.tile([P, RCOL], F32)
            nc.vector.tensor_tensor(out=dinv2[:], in0=dinv[:], in1=dinv[:],
                                    op=mybir.AluOpType.mult)
            z2 = sb.tile([P, RCOL, 4], F32)
            for d in range(4):
                nc.vector.tensor_tensor(out=z2[:, :, d], in0=aggr[:, :, d],
                                        in1=dinv[:], op=mybir.AluOpType.mult)
                t = sb.tile([P, RCOL], F32, tag="k3tmp")
                nc.vector.tensor_tensor(out=t[:], in0=h1t[:, :, d],
                                        in1=dinv2[:], op=mybir.AluOpType.mult)
                nc.vector.tensor_tensor(out=z2[:, :, d], in0=z2[:, :, d],
                                        in1=t[:], op=mybir.AluOpType.add)
            h2t = sb.tile([P, RCOL, 4], F32)
            for dout in range(4):
                # acc = z2_0*W2[0,dout]
                nc.vector.tensor_scalar(
                    out=h2t[:, :, dout], in0=z2[:, :, 0],
                    scalar1=wb[:, 8 + 0 * 4 + dout:8 + 0 * 4 + dout + 1],
                    scalar2=None, op0=mybir.AluOpType.mult)
                for din in range(1, 4):
                    nc.vector.scalar_tensor_tensor(
                        out=h2t[:, :, dout], in0=z2[:, :, din],
                        scalar=wb[:, 8 + din * 4 + dout:8 + din * 4 + dout + 1],
                        in1=h2t[:, :, dout],
                        op0=mybir.AluOpType.mult, op1=mybir.AluOpType.add)
                nc.vector.tensor_scalar(
                    out=h2t[:, :, dout], in0=h2t[:, :, dout],
                    scalar1=wb[:, 24 + dout:25 + dout], scalar2=None,
                    op0=mybir.AluOpType.add)
            nc.sync.dma_start(out=h2o[:], in_=h2t[:])
    nc.compile()
    return nc


def _build_k4():
    """logits per slot: dot(g3[slot,:], h2own[dst(slot),:]) with dense
    dst-expansion per class block."""
    nc = bacc.Bacc(None)
    g3 = nc.dram_tensor("g3", [P, C1 * 4], BF16, kind="ExternalInput")
    h2r = nc.dram_tensor("h2r", [P, RCOL * 4], F32, kind="ExternalInput")
    lg = nc.dram_tensor("lg", [P, C1], F32, kind="ExternalOutput")
    with tile.TileContext(nc) as tc:
        with (tc.tile_pool(name="sbuf", bufs=1) as sb,
              tc.tile_pool(name="stream", bufs=2) as st):
            colbase = 0
            for (r0, r1, S) in CLS:
                n = r1 - r0
                npp = n // P
                mchunk = max(1, CHUNK // S)
                for m0 in range(0, npp, mchunk):
                    mm = min(mchunk, npp - m0)
                    gb = st.tile([P, mchunk * S * 4], BF16, tag="g3in")
                    gfc = st.tile([P, mchunk * S * 4], F32, tag="g3f")
                    c0 = colbase + m0 * S
                    nc.sync.dma_start(out=gb[:, : mm * S * 4],
                                      in_=g3[:, c0 * 4:(c0 + mm * S) * 4])
                    # expand h2 of the mm nodes across their S slots (bf16);
                    # class rank mapping: rank = r0 + p*npp + (m0+m)
                    ex = st.tile([P, mchunk * S * 4], BF16, tag="expand")
                    exv = ex[:, : mm * S * 4].rearrange(
                        "p (m s d) -> p m s d", m=mm, s=S)
                    h2c = st.tile([P, mchunk, 1, 4], F32, tag="h2c")
                    src = bass.AP(h2r[:].tensor,
                                  h2r[:].offset + (r0 + m0) * 4,
                                  [[npp * 4, P], [4, mm], [1, 4]])
                    nc.sync.dma_start(out=h2c[:, : mm, :, :], in_=src)
                    nc.vector.tensor_copy(out=exv[:, :, 0:1, :],
                                          in_=h2c[:, : mm, :, :])
                    step = 1
                    while step < S:
                        w = min(step, S - step)
                        nc.vector.tensor_copy(out=exv[:, :, step:step + w, :],
                                              in_=exv[:, :, 0:w, :])
                        step += w
                    # dot: bf16*bf16 -> f32, then sum over d
                    nc.vector.tensor_tensor(out=gfc[:, : mm * S * 4],
                                            in0=gb[:, : mm * S * 4],
                                            in1=ex[:, : mm * S * 4],
                                            op=mybir.AluOpType.mult)
                    gv = gfc[:, : mm * S * 4].rearrange(
                        "p (c d) -> p c d", d=4)
                    nc.vector.tensor_tensor(out=gv[:, :, 0:2], in0=gv[:, :, 0:2],
                                            in1=gv[:, :, 2:4],
                                            op=mybir.AluOpType.add)
                    nc.vector.tensor_tensor(out=gv[:, :, 0:1], in0=gv[:, :, 0:1],
                                            in1=gv[:, :, 1:2],
                                            op=mybir.AluOpType.add)
                    lt = st.tile([P, mchunk * S, 1], F32, tag="lout")
                    nc.vector.tensor_copy(out=lt[:, : mm * S, :],
                                          in_=gv[:, :, 0:1])
                    nc.sync.dma_start(out=lg[:, c0:c0 + mm * S],
                                      in_=lt[:, : mm * S, :])
                colbase += npp * S
    nc.compile()
    return nc


_KERNELS = {}


def _get_kernels():
    if not _KERNELS:
        _KERNELS["k1"] = _build_k1()
        _KERNELS["k2"] = _build_k2()
        _KERNELS["k3"] = _build_k3()
        _KERNELS["k4"] = _build_k4()
    return _KERNELS


def _run(nc, in_maps):
    res = run_bass_kernel_spmd(nc, in_maps, list(range(N_CORES)),
                               trace=_TRACE)
    if res.exec_time_ns is not None:
        LAST_EXEC_NS.append(res.exec_time_ns)
    return res.results


def kernel(x, edge_index, W1, b1, W2, b2):
    x = np.asarray(x).reshape(-1).astype(np.float32)
    edge_index = np.asarray(edge_index)
    src = edge_index[0].astype(np.int64)
    dst = edge_index[1].astype(np.int64)
    import ml_dtypes

    LAST_EXEC_NS.clear()
    ks = _get_kernels()

    deg = np.bincount(dst, minlength=N_NODES).astype(np.int64)

    # ---- host index prep per core ----
    order_e = np.argsort(dst, kind="stable")
    dst_s = dst[order_e]
    src_s = src[order_e]
    bounds = np.searchsorted(dst_s, np.arange(N_CORES + 1) * OWN)

    x_pad = np.zeros(N_CORES * OWN_PAD, dtype=np.float32)
    deg_pad = np.zeros(N_CORES * OWN_PAD, dtype=np.int32)
    x_pad[:N_NODES] = x
    deg_pad[:N_NODES] = deg

    wvec = np.concatenate([
        np.asarray(W1, np.float32).reshape(-1),
        np.asarray(b1, np.float32).reshape(-1),
        np.asarray(W2, np.float32).reshape(-1),
        np.asarray(b2, np.float32).reshape(-1),
    ]).astype(np.float32)
    assert wvec.shape == (28,)

    cores = []
    for c in range(N_CORES):
        lo, hi = bounds[c], bounds[c + 1]
        sd = dst_s[lo:hi] - c * OWN      # local dst ids (sorted)
        ss = src_s[lo:hi]
        eid = order_e[lo:hi]

        d_own = np.zeros(OWN_PAD, dtype=np.int64)
        d_own[:OWN] = deg[c * OWN:(c + 1) * OWN]
        rank_order = np.argsort(-d_own, kind="stable")
        rank_of = np.empty(OWN_PAD, dtype=np.int64)
        rank_of[rank_order] = np.arange(OWN_PAD)

        dsr = np.sort(-d_own) * -1
        assert dsr[0] <= 64, f"deg {dsr[0]} exceeds max class"
        assert dsr[CLS[0][1]] <= 32, "class-32 boundary violated"
        assert dsr[CLS[1][1]] <= 16, "class-16 boundary violated"

        # flat-grid base address per rank
        base = np.empty(OWN_PAD, dtype=np.int64)
        colbase = 0
        for (r0, r1, S) in CLS:
            n = r1 - r0
            npp = n // P
            rr = np.arange(r0, r1)
            p = (rr - r0) // npp
            m = (rr - r0) % npp
            base[rr] = p * C1 + colbase + m * S
            colbase += npp * S

        # within-node edge position j (dst-sorted => runs contiguous)
        first = np.ones(len(sd), dtype=bool)
        first[1:] = sd[1:] != sd[:-1]
        runstart = np.maximum.accumulate(
            np.where(first, np.arange(len(sd)), 0))
        j = np.arange(len(sd)) - runstart

        slot = base[rank_of[sd]] + j
        src_slot = np.full(TOT_SLOTS, N_NODES, dtype=np.int64)
        src_slot[slot] = ss
        edge_of_slot = np.full(TOT_SLOTS, -1, dtype=np.int64)
        edge_of_slot[slot] = eid

        own_ids = c * OWN + rank_order  # rank -> original id (pad ids >= OWN are fake)
        own_valid = rank_order < OWN

        cores.append(dict(
            src_slot=src_slot, edge_of_slot=edge_of_slot,
            own_ids=own_ids, own_valid=own_valid,
            xr=x[np.minimum(own_ids, N_NODES - 1)].astype(np.float32)
            * own_valid,
            degr=(deg[np.minimum(own_ids, N_NODES - 1)] * own_valid
                  ).astype(np.int32),
        ))

    # ---- launch 1: u = x * rsqrt(deg+1) over all nodes (linear shards) ----
    in1 = [{"x": x_pad[c * OWN_PAD:(c + 1) * OWN_PAD].reshape(P, RCOL),
            "deg": deg_pad[c * OWN_PAD:(c + 1) * OWN_PAD].reshape(P, RCOL)}
           for c in range(N_CORES)]
    r1 = _run(ks["k1"], in1)
    u_full = np.zeros(N_CORES * OWN_PAD + 1, dtype=np.float32)
    for c in range(N_CORES):
        u_full[c * OWN_PAD:(c + 1) * OWN_PAD] = r1[c]["u"].reshape(-1)
    u_full[N_NODES:] = 0.0
    u_pad = np.zeros(N_NODES + 1, dtype=np.float32)
    u_pad[:N_NODES] = u_full[:N_NODES]

    # ---- launch 2: layer 1 ----
    in2 = []
    for c in range(N_CORES):
        g1 = u_pad[np.minimum(cores[c]["src_slot"], N_NODES)]
        in2.append({"g1": g1.reshape(P, C1).astype(ml_dtypes.bfloat16),
                    "xr": cores[c]["xr"].reshape(P, RCOL),
                    "degr": cores[c]["degr"].reshape(P, RCOL),
                    "wvec": wvec})
    r2 = _run(ks["k2"], in2)
    h1u_full = np.zeros((N_NODES + 1, 4), dtype=np.float32)
    h1r_per_core = []
    for c in range(N_CORES):
        h1u_r = r2[c]["h1u"].reshape(OWN_PAD, 4)
        h1r_per_core.append(r2[c]["h1o"])
        ov = cores[c]["own_valid"]
        h1u_full[cores[c]["own_ids"][ov]] = h1u_r[ov]

    # ---- launch 3: layer 2 ----
    in3 = []
    for c in range(N_CORES):
        g2 = h1u_full[np.minimum(cores[c]["src_slot"], N_NODES)]
        in3.append({"g2": g2.reshape(P, C1 * 4).astype(ml_dtypes.bfloat16),
                    "h1r": h1r_per_core[c],
                    "degr": cores[c]["degr"].reshape(P, RCOL),
                    "wvec": wvec})
    r3 = _run(ks["k3"], in3)
    h2_full = np.zeros((N_NODES + 1, 4), dtype=np.float32)
    h2r_per_core = []
    for c in range(N_CORES):
        h2_r = r3[c]["h2o"].reshape(OWN_PAD, 4)
        h2r_per_core.append(r3[c]["h2o"])
        ov = cores[c]["own_valid"]
        h2_full[cores[c]["own_ids"][ov]] = h2_r[ov]

    # ---- launch 4: logits ----
    in4 = []
    for c in range(N_CORES):
        g3 = h2_full[np.minimum(cores[c]["src_slot"], N_NODES)]
        in4.append({"g3": g3.reshape(P, C1 * 4).astype(ml_dtypes.bfloat16),
                    "h2r": h2r_per_core[c]})
    r4 = _run(ks["k4"], in4)

    logits = np.zeros(N_EDGES, dtype=np.float32)
    for c in range(N_CORES):
        lg = r4[c]["lg"].reshape(-1)
        es = cores[c]["edge_of_slot"]
        valid = es >= 0
        logits[es[valid]] = lg[valid]
    return logits
